# revision 22
# baseline (speedup 1.0000x reference)
"""CQAttention Trainium2 kernel.

Reference computation per batch b (C:[D,Lc], Q:[D,Lq], D=128, Lc=2048, Lq=512):
    Ct = C^T, Qt = Q^T
    S  = Ct@w4C + (Qt@w4Q)^T + (Ct*w4mlu)@Qt^T + bias        [Lc, Lq]
    S1 = softmax_q(S + NEG*(1-qmask))                         (over Lq)
    S2 = softmax_c(S + NEG*(1-cmask))                         (over Lc)
    A  = S1 @ Qt                                              [Lc, D]
    B  = S1 @ (S2^T @ Ct)     (right-assoc of (S1 S2^T) Ct)   [Lc, D]
    out= transpose(concat([Ct, A, Ct*A, Ct*B], -1))           [4D, Lc]

Kernel strategy (pure data parallel over batch: 16 batches / 8 cores):
  - S is never materialized: two matmul families compute S(sub2-part) in
    [c-part,q-free] and [q-part,c-free] layouts straight into PSUM; ScalarE
    exp() reads PSUM with a per-partition bias AP folding the softmax-relevant
    affine terms (terms constant along the softmax axis cancel). sub2 uses
    Qw = Q*w4mlu, so C needs no scaled copy (C^T diag(w) Q == C^T (w * Q)).
  - Tile-pipelined phases: each E2=[c,q] exp tile immediately feeds
    R += E2^T@Ct (whole-bank single PSUM accumulation group across all 4
    q-tiles x 16 c-tiles) plus a paired N=1 ones-matmul accumulating the S2
    normalizer in its own bank; PE(S), ACT(exp), PE(R) overlap per c-tile.
  - The S1 normalizer (colsum over q of E1T) is a ones[128,128]-weights
    matmul: every PSUM partition row carries the same colsum, so the
    reciprocal is directly partition-broadcast in SBUF - no DRAM bounce.
  - PSUM pools are phase-private (8 banks: 2 E2-S/transpose/cq, 2 E1T-S/csp,
    2 pa/pb, 1 rp, 1 s2sum) so batch b+1's front (Tile priority boost) never
    rotates behind batch b's tail allocations.
  - PE transposes run as 4-per-bank single accumulation groups (f32r identity,
    1.5 cyc/row) evacuated by one 512-row copy each.
"""

import numpy as np
from contextlib import ExitStack, nullcontext

import concourse.bass as bass
import concourse.mybir as mybir
import concourse.tile as tile
from concourse import bacc
from concourse.bass_utils import run_bass_kernel_spmd
from concourse.masks import make_identity

F32 = mybir.dt.float32
F32R = mybir.dt.float32r
I32 = mybir.dt.int32
AF = mybir.ActivationFunctionType
ALU = mybir.AluOpType
BF16 = mybir.dt.bfloat16

B, D, LC, LQ = 16, 128, 2048, 512
NCORES = 8
BL = B // NCORES          # batches per core
NEG = -1e30
NCT = LC // 128           # 16 c-tiles
NQT = LQ // 128           # 4 q-tiles
NCJ = LC // 512           # 4 c-chunks (free-dim)
HIPRI_OFF = 280


def _build_nc():
    nc = bacc.Bacc("TRN2", target_bir_lowering=False)
    Ci = nc.dram_tensor("C", [BL, D, LC], F32, kind="ExternalInput")
    Qi = nc.dram_tensor("Q", [BL, D, LQ], F32, kind="ExternalInput")
    CMi = nc.dram_tensor("Cmask", [BL, LC], I32, kind="ExternalInput")
    QMi = nc.dram_tensor("Qmask", [BL, LQ], I32, kind="ExternalInput")
    w4C = nc.dram_tensor("w4C", [D, 1], F32, kind="ExternalInput")
    w4Q = nc.dram_tensor("w4Q", [D, 1], F32, kind="ExternalInput")
    w4mlu = nc.dram_tensor("w4mlu", [1, 1, D], F32, kind="ExternalInput")
    biasi = nc.dram_tensor("bias", [1], F32, kind="ExternalInput")
    out = nc.dram_tensor("out", [BL, 4 * D, LC], F32, kind="ExternalOutput")

    with tile.TileContext(nc) as tc, ExitStack() as ctx:
        const = ctx.enter_context(tc.tile_pool(name="const", bufs=1))
        sb2 = ctx.enter_context(tc.tile_pool(name="sb2", bufs=2))
        sb4 = ctx.enter_context(tc.tile_pool(name="sb4", bufs=4))
        # PSUM: 8 banks, phase-private pools (see module docstring)
        ps_7 = ctx.enter_context(tc.tile_pool(name="ps_7", bufs=2, space="PSUM"))
        ps_8 = ctx.enter_context(tc.tile_pool(name="ps_8", bufs=2, space="PSUM"))
        ps_ab = ctx.enter_context(tc.tile_pool(name="ps_ab", bufs=2, space="PSUM"))
        ps_r = ctx.enter_context(tc.tile_pool(name="ps_r", bufs=1, space="PSUM"))
        ps_cs = ctx.enter_context(tc.tile_pool(name="ps_cs", bufs=1, space="PSUM"))

        # ---- constants; identity first (Pool), then SWDGE w-consts so the
        #      weight loads don't sit behind identity building on Pool ----
        ident0 = const.tile([D, D], F32, name="ident0")
        make_identity(nc, ident0)
        identR = const.tile([D, D], F32R, name="identR")
        nc.vector.tensor_copy(identR, ident0)
        ones_f = const.tile([D, D], F32, name="ones_f")
        nc.vector.memset(ones_f, 1.0)
        onesR128 = const.tile([D, D], F32R, name="onesR128")
        nc.vector.tensor_copy(onesR128, ones_f)
        ones_colB = const.tile([D, 1], BF16, name="ones_colB")
        nc.vector.tensor_copy(ones_colB, ones_f[:, 0:1])
        wmlu_sb = const.tile([D, 1], F32, name="wmlu_sb")
        nc.gpsimd.dma_start(out=wmlu_sb, in_=w4mlu.ap().rearrange("a b d -> d (a b)"))
        w4C_sb = const.tile([D, 1], F32, name="w4C_sb")
        nc.gpsimd.dma_start(out=w4C_sb, in_=w4C[:, :])
        w4Q_sb = const.tile([D, 1], F32, name="w4Q_sb")
        nc.gpsimd.dma_start(out=w4Q_sb, in_=w4Q[:, :])
        bias_bc = const.tile([D, 1], F32, name="bias_bc")
        nc.scalar.dma_start(out=bias_bc, in_=biasi.ap().partition_broadcast(D))

        for b in range(BL):
            with (tc.high_priority(HIPRI_OFF) if b > 0 else nullcontext()):
                # ---- loads: Q, C-half0, masks, C-half1 ----
                Q_sb = sb2.tile([D, LQ], F32R, name="Q_sb")
                nc.sync.dma_start(out=Q_sb, in_=Qi[b, :, :].bitcast(F32R))
                C_sb = sb2.tile([D, LC], F32, name="C_sb")
                nc.sync.dma_start(out=C_sb[:, 0:1024], in_=Ci[b, :, 0:1024])
                cm_i = sb2.tile([128, NCT], I32, name="cm_i")
                nc.sync.dma_start(out=cm_i, in_=CMi[b, :].rearrange("(i p) -> p i", p=128))
                nc.sync.dma_start(out=C_sb[:, 1024:2048], in_=Ci[b, :, 1024:2048])
                qm_i = sb2.tile([128, NQT], I32, name="qm_i")
                nc.sync.dma_start(out=qm_i, in_=QMi[b, :].rearrange("(i p) -> p i", p=128))

                # ---- small prep ----
                Qw = sb2.tile([D, LQ], F32R, name="Qw")
                nc.vector.tensor_scalar_mul(Qw, Q_sb, wmlu_sb[:, 0:1])
                # mask -> NEG*(1-m):  m*(-NEG) + NEG
                cneg = sb2.tile([128, NCT], F32, name="cneg")
                nc.vector.tensor_scalar(cneg, cm_i, -NEG, NEG, op0=ALU.mult, op1=ALU.add)
                qneg = sb2.tile([128, NQT], F32, name="qneg")
                nc.vector.tensor_scalar(qneg, qm_i, -NEG, NEG, op0=ALU.mult, op1=ALU.add)
                # f32r matmul operands must be pre-rounded to f32r in SBUF
                CR = sb2.tile([D, LC], F32R, name="CR")
                for cj in range(NCJ):
                    eng = nc.vector if cj < 2 else nc.gpsimd
                    eng.tensor_copy(CR[:, cj * 512 : (cj + 1) * 512],
                                    C_sb[:, cj * 512 : (cj + 1) * 512])

                # c0[c] = sum_d C[d,c] w4C[d]; q1[q] = sum_d Q[d,q] w4Q[d]
                cq_p = ps_7.tile([128, NCT + NQT], F32, name="s7")
                for qi in range(NQT):
                    nc.tensor.matmul(cq_p[:, NCT + qi : NCT + qi + 1],
                                     Q_sb.bitcast(F32)[:, qi * 128 : (qi + 1) * 128],
                                     w4Q_sb, start=True, stop=True)
                for ci in range(NCT):
                    nc.tensor.matmul(cq_p[:, ci : ci + 1],
                                     C_sb[:, ci * 128 : (ci + 1) * 128],
                                     w4C_sb, start=True, stop=True)
                bias_c = sb2.tile([128, NCT], F32, name="bias_c")
                nc.vector.tensor_tensor(bias_c[:, 0 : NCT // 2], cq_p[:, 0 : NCT // 2],
                                        cneg[:, 0 : NCT // 2], ALU.add)
                nc.vector.tensor_tensor(bias_c[:, NCT // 2 : NCT], cq_p[:, NCT // 2 : NCT],
                                        cneg[:, NCT // 2 : NCT], ALU.add)
                bias_q0 = sb2.tile([128, NQT], F32, name="bias_q0")
                nc.vector.tensor_tensor(bias_q0, cq_p[:, NCT : NCT + NQT], qneg, ALU.add)
                bias_q = sb2.tile([128, NQT], F32, name="bias_q")
                nc.vector.tensor_scalar_add(bias_q, bias_q0, bias_bc[:, 0:1])

                # ---- transposes: 4 per bank-wide group, in the s2pp bank
                #      (sequential lifetimes: transposes close before s2pp opens)
                Qt_sb = sb2.tile([128, NQT, 128], F32R, name="Qt_sb")
                tpq = ps_cs.tile([128, NQT, 128], F32R, name="cs")
                for qi in range(NQT):
                    nc.tensor.matmul(tpq[:, qi, :], Q_sb[:, qi * 128 : (qi + 1) * 128],
                                     identR, is_transpose=True,
                                     start=(qi == 0), stop=(qi == NQT - 1))
                nc.vector.tensor_copy(Qt_sb, tpq)
                Ct_sb = sb2.tile([128, NCT, 128], BF16, name="Ct_sb")
                for g in range(NCT // 4):
                    tp = ps_cs.tile([128, 4, 128], F32R, name="cs")
                    for k in range(4):
                        ci = 4 * g + k
                        nc.tensor.matmul(tp[:, k, :], CR[:, ci * 128 : (ci + 1) * 128],
                                         identR, is_transpose=True,
                                         start=(k == 0), stop=(k == 3))
                    if g % 2 == 0:
                        nc.vector.tensor_copy(Ct_sb[:, 4 * g : 4 * g + 4, :], tp)
                    else:
                        nc.scalar.copy(Ct_sb[:, 4 * g : 4 * g + 4, :], tp)

                # ---- E2 = exp(S + c-terms) in [c-part, q-free], fused with
                #      R[q,d] += E2[c,q]^T Ct[c,d] and s2sum[q] += colsum ----
                E2 = sb2.tile([128, NCT, LQ], BF16, name="E2")
                rp = ps_r.tile([128, NQT, 128], F32, name="rp")
                s2pp = ps_cs.tile([128, NQT], F32, name="cs")
                for ci in range(NCT):
                    sp = ps_7.tile([128, LQ], F32, name="s7")
                    nc.tensor.matmul(sp, CR[:, ci * 128 : (ci + 1) * 128], Qw,
                                     start=True, stop=True)
                    nc.scalar.activation(E2[:, ci, :], sp, AF.Exp,
                                         bias=bias_c[:, ci : ci + 1], scale=1.0)
                    for qi in range(NQT):
                        lhs = E2[:, ci, qi * 128 : (qi + 1) * 128]
                        first = ci == 0 and qi == 0
                        last = ci == NCT - 1 and qi == NQT - 1
                        nc.tensor.matmul(rp[:, qi, :], lhs, Ct_sb[:, ci, :],
                                         start=first, stop=last)
                        nc.tensor.matmul(s2pp[:, qi : qi + 1], lhs, ones_colB,
                                         start=first, stop=last)

            # ---- phase 8: E1T = exp(S^T + q-terms) in [q-part, c-free];
            #      per 512-col c-chunk: colsum -> broadcast rcp, A^T, B^T ----
            # R normalize sits here in program order so it doesn't head-of-line
            # block the DVE queue while the rp accumulation is still open.
            rs2 = sb2.tile([128, NQT], F32, name="rs2")
            nc.vector.reciprocal(rs2, s2pp)
            R_sb = sb2.tile([128, NQT, 128], F32R, name="R_sb")
            for qi in range(NQT):
                nc.vector.tensor_scalar_mul(R_sb[:, qi, :], rp[:, qi, :],
                                            rs2[:, qi : qi + 1])
            # C pass-through store: mid-kernel DMA lull, away from loads/tail
            nc.scalar.dma_start(out=out[b, 0:128, :], in_=C_sb)

            E1T = sb2.tile([128, NQT, LC], F32R, name="E1T")

            def consume(cj):
                sl = slice(cj * 512, (cj + 1) * 512)
                csp = ps_ab.tile([128, 512], F32, name="pab")
                for qi in range(NQT):
                    nc.tensor.matmul(csp, onesR128, E1T[:, qi, sl],
                                     start=(qi == 0), stop=(qi == NQT - 1))
                rcp_c = sb4.tile([128, 512], F32, name="rcp_c")
                nc.vector.reciprocal(rcp_c, csp)
                last = b == BL - 1
                pa = ps_ab.tile([128, 512], F32, name="pab")
                pb = ps_ab.tile([128, 512], F32, name="pab")
                # last batch: pb first - its Bt/CB chain is the tail's long pole
                for p, t in ((pb, R_sb), (pa, Qt_sb)) if last else ((pa, Qt_sb), (pb, R_sb)):
                    for qi in range(NQT):
                        nc.tensor.matmul(p, t[:, qi, :], E1T[:, qi, sl],
                                         start=(qi == 0), stop=(qi == NQT - 1))
                ACB = sb2.tile([128, 3, 512], F32, name="ACB")
                Bt_t = sb2.tile([128, 512], F32, name="Bt_t")
                if last:
                    # 256-wide half-chunks: halves the elementwise+store latency
                    for h in range(2):
                        hs = slice(h * 256, (h + 1) * 256)
                        hsl = slice(cj * 512 + h * 256, cj * 512 + (h + 1) * 256)
                        nc.vector.tensor_tensor(Bt_t[:, hs], pb[:, hs], rcp_c[:, hs],
                                                ALU.mult)
                        nc.gpsimd.tensor_tensor(ACB[:, 2, hs], C_sb[:, hsl],
                                                Bt_t[:, hs], ALU.mult)
                        nc.vector.tensor_tensor(ACB[:, 0, hs], pa[:, hs], rcp_c[:, hs],
                                                ALU.mult)
                        nc.vector.tensor_tensor(ACB[:, 1, hs], C_sb[:, hsl],
                                                ACB[:, 0, hs], ALU.mult)
                        nc.sync.dma_start(out=out[b, 128:256, hsl], in_=ACB[:, 0, hs])
                        nc.sync.dma_start(out=out[b, 384:512, hsl], in_=ACB[:, 2, hs])
                        nc.sync.dma_start(out=out[b, 256:384, hsl], in_=ACB[:, 1, hs])
                else:
                    nc.vector.tensor_tensor(ACB[:, 0, :], pa, rcp_c, ALU.mult)
                    nc.vector.tensor_tensor(Bt_t, pb, rcp_c, ALU.mult)
                    nc.vector.tensor_tensor(ACB[:, 1, :], C_sb[:, sl], ACB[:, 0, :],
                                            ALU.mult)
                    nc.gpsimd.tensor_tensor(ACB[:, 2, :], C_sb[:, sl], Bt_t, ALU.mult)
                    q = nc.sync if cj % 2 == 0 else nc.scalar
                    q.dma_start(
                        out=out[b, 128:512, sl].rearrange("(r p) c -> p r c", p=128),
                        in_=ACB,
                    )

            for cj in range(NCJ):
                sl = slice(cj * 512, (cj + 1) * 512)
                for qi in range(NQT):
                    sp = ps_8.tile([128, 512], F32, name="s8")
                    nc.tensor.matmul(sp, Qw[:, qi * 128 : (qi + 1) * 128],
                                     CR[:, sl], start=True, stop=True)
                    nc.scalar.activation(E1T[:, qi, sl], sp, AF.Exp,
                                         bias=bias_q[:, qi : qi + 1], scale=1.0)
                if cj > 0:
                    consume(cj - 1)
            consume(NCJ - 1)

    nc.finalize()
    return nc


_NC = None


def _get_nc():
    global _NC
    if _NC is None:
        _NC = _build_nc()
    return _NC


def kernel(C, Q, Cmask, Qmask, w4C, w4Q, w4mlu, bias, _trace=False):
    C = np.ascontiguousarray(np.asarray(C, dtype=np.float32))
    Q = np.ascontiguousarray(np.asarray(Q, dtype=np.float32))
    Cmask = np.ascontiguousarray(np.asarray(Cmask, dtype=np.int32))
    Qmask = np.ascontiguousarray(np.asarray(Qmask, dtype=np.int32))
    w4C = np.ascontiguousarray(np.asarray(w4C, dtype=np.float32))
    w4Q = np.ascontiguousarray(np.asarray(w4Q, dtype=np.float32))
    w4mlu = np.ascontiguousarray(np.asarray(w4mlu, dtype=np.float32))
    bias = np.ascontiguousarray(np.asarray(bias, dtype=np.float32))

    nc = _get_nc()
    in_maps = []
    for i in range(NCORES):
        s = slice(i * BL, (i + 1) * BL)
        in_maps.append({
            "C": C[s], "Q": Q[s], "Cmask": Cmask[s], "Qmask": Qmask[s],
            "w4C": w4C, "w4Q": w4Q, "w4mlu": w4mlu, "bias": bias,
        })
    res = run_bass_kernel_spmd(nc, in_maps, core_ids=list(range(NCORES)),
                               trace=_trace)
    out = np.concatenate([r["out"] for r in res.results], axis=0)
    if _trace:
        kernel._last_results = res
    return out


# revision 29
# speedup vs baseline: 1.0854x; 1.0854x over previous
"""CQAttention Trainium2 kernel.

Reference computation per batch b (C:[D,Lc], Q:[D,Lq], D=128, Lc=2048, Lq=512):
    Ct = C^T, Qt = Q^T
    S  = Ct@w4C + (Qt@w4Q)^T + (Ct*w4mlu)@Qt^T + bias        [Lc, Lq]
    S1 = softmax_q(S + NEG*(1-qmask))                         (over Lq)
    S2 = softmax_c(S + NEG*(1-cmask))                         (over Lc)
    A  = S1 @ Qt                                              [Lc, D]
    B  = S1 @ (S2^T @ Ct)     (right-assoc of (S1 S2^T) Ct)   [Lc, D]
    out= transpose(concat([Ct, A, Ct*A, Ct*B], -1))           [4D, Lc]

Kernel strategy (pure data parallel over batch: 16 batches / 8 cores):
  - S is never materialized: two matmul families compute S(sub2-part) in
    [c-part,q-free] and [q-part,c-free] layouts straight into PSUM; ScalarE
    exp() reads PSUM with a per-partition bias AP folding the softmax-relevant
    affine terms (terms constant along the softmax axis cancel). sub2 uses
    Qw = Q*w4mlu, so C needs no scaled copy (C^T diag(w) Q == C^T (w * Q)).
  - Tile-pipelined phases: each E2=[c,q] exp tile immediately feeds
    R += E2^T@Ct (whole-bank single PSUM accumulation group across all 4
    q-tiles x 16 c-tiles) plus a paired N=1 ones-matmul accumulating the S2
    normalizer in its own bank; PE(S), ACT(exp), PE(R) overlap per c-tile.
  - The S1 normalizer (colsum over q of E1T) is a ones[128,128]-weights
    matmul: every PSUM partition row carries the same colsum, so the
    reciprocal is directly partition-broadcast in SBUF - no DRAM bounce.
  - PSUM pools are phase-private (8 banks: 2 E2-S/transpose/cq, 2 E1T-S/csp,
    2 pa/pb, 1 rp, 1 s2sum) so batch b+1's front (Tile priority boost) never
    rotates behind batch b's tail allocations.
  - PE transposes run as 4-per-bank single accumulation groups (f32r identity,
    1.5 cyc/row) evacuated by one 512-row copy each.
"""

import numpy as np
from contextlib import ExitStack, nullcontext

import concourse.bass as bass
import concourse.mybir as mybir
import concourse.tile as tile
from concourse import bacc
from concourse.bass_utils import run_bass_kernel_spmd
from concourse.masks import make_identity

F32 = mybir.dt.float32
F32R = mybir.dt.float32r
I32 = mybir.dt.int32
AF = mybir.ActivationFunctionType
ALU = mybir.AluOpType
BF16 = mybir.dt.bfloat16

B, D, LC, LQ = 16, 128, 2048, 512
NCORES = 8
BL = B // NCORES          # batches per core
NEG = -1e30
NCT = LC // 128           # 16 c-tiles
NQT = LQ // 128           # 4 q-tiles
NCJ = LC // 512           # 4 c-chunks (free-dim)
HIPRI_OFF = 280
P8_BOOST = 40


def _build_nc():
    nc = bacc.Bacc("TRN2", target_bir_lowering=False)
    Ci = nc.dram_tensor("C", [BL, D, LC], F32, kind="ExternalInput")
    Qi = nc.dram_tensor("Q", [BL, D, LQ], F32, kind="ExternalInput")
    CMi = nc.dram_tensor("Cmask", [BL, LC], I32, kind="ExternalInput")
    QMi = nc.dram_tensor("Qmask", [BL, LQ], I32, kind="ExternalInput")
    w4C = nc.dram_tensor("w4C", [D, 1], F32, kind="ExternalInput")
    w4Q = nc.dram_tensor("w4Q", [D, 1], F32, kind="ExternalInput")
    w4mlu = nc.dram_tensor("w4mlu", [1, 1, D], F32, kind="ExternalInput")
    biasi = nc.dram_tensor("bias", [1], F32, kind="ExternalInput")
    out = nc.dram_tensor("out", [BL, 4 * D, LC], F32, kind="ExternalOutput")

    with tile.TileContext(nc) as tc, ExitStack() as ctx:
        const = ctx.enter_context(tc.tile_pool(name="const", bufs=1))
        sb2 = ctx.enter_context(tc.tile_pool(name="sb2", bufs=2))
        sb4 = ctx.enter_context(tc.tile_pool(name="sb4", bufs=4))
        # PSUM: 8 banks, phase-private pools (see module docstring)
        ps_7 = ctx.enter_context(tc.tile_pool(name="ps_7", bufs=2, space="PSUM"))
        ps_8 = ctx.enter_context(tc.tile_pool(name="ps_8", bufs=2, space="PSUM"))
        ps_ab = ctx.enter_context(tc.tile_pool(name="ps_ab", bufs=2, space="PSUM"))
        ps_r = ctx.enter_context(tc.tile_pool(name="ps_r", bufs=1, space="PSUM"))
        ps_cs = ctx.enter_context(tc.tile_pool(name="ps_cs", bufs=1, space="PSUM"))

        # ---- constants; identity first (Pool), then SWDGE w-consts so the
        #      weight loads don't sit behind identity building on Pool ----
        ident0 = const.tile([D, D], F32, name="ident0")
        make_identity(nc, ident0)
        identR = const.tile([D, D], F32R, name="identR")
        nc.vector.tensor_copy(identR, ident0)
        ones_f = const.tile([D, D], F32, name="ones_f")
        nc.vector.memset(ones_f, 1.0)
        onesR128 = const.tile([D, D], F32R, name="onesR128")
        nc.vector.tensor_copy(onesR128, ones_f)
        ones_colB = const.tile([D, 1], BF16, name="ones_colB")
        nc.vector.tensor_copy(ones_colB, ones_f[:, 0:1])
        wmlu_sb = const.tile([D, 1], F32, name="wmlu_sb")
        nc.gpsimd.dma_start(out=wmlu_sb, in_=w4mlu.ap().rearrange("a b d -> d (a b)"))
        w4C_sb = const.tile([D, 1], F32, name="w4C_sb")
        nc.gpsimd.dma_start(out=w4C_sb, in_=w4C[:, :])
        w4Q_sb = const.tile([D, 1], F32, name="w4Q_sb")
        nc.gpsimd.dma_start(out=w4Q_sb, in_=w4Q[:, :])
        bias_bc = const.tile([D, 1], F32, name="bias_bc")
        nc.scalar.dma_start(out=bias_bc, in_=biasi.ap().partition_broadcast(D))

        for b in range(BL):
            with (tc.high_priority(HIPRI_OFF) if b > 0 else nullcontext()):
                # ---- loads: Q, C-half0, masks, C-half1 ----
                Q_sb = sb2.tile([D, LQ], F32R, name="Q_sb")
                nc.sync.dma_start(out=Q_sb, in_=Qi[b, :, :].bitcast(F32R))
                C_sb = sb2.tile([D, LC], F32, name="C_sb")
                nc.sync.dma_start(out=C_sb[:, 0:1024], in_=Ci[b, :, 0:1024])
                cm_i = sb2.tile([128, NCT], I32, name="cm_i")
                nc.sync.dma_start(out=cm_i, in_=CMi[b, :].rearrange("(i p) -> p i", p=128))
                qm_i = sb2.tile([128, NQT], I32, name="qm_i")
                nc.sync.dma_start(out=qm_i, in_=QMi[b, :].rearrange("(i p) -> p i", p=128))
                nc.sync.dma_start(out=C_sb[:, 1024:2048], in_=Ci[b, :, 1024:2048])

                # ---- small prep ----
                Qw = sb2.tile([D, LQ], F32R, name="Qw")
                nc.vector.tensor_scalar_mul(Qw, Q_sb, wmlu_sb[:, 0:1])
                # mask -> NEG*(1-m):  m*(-NEG) + NEG
                cneg = sb2.tile([128, NCT], F32, name="cneg")
                nc.vector.tensor_scalar(cneg, cm_i, -NEG, NEG, op0=ALU.mult, op1=ALU.add)
                qneg = sb2.tile([128, NQT], F32, name="qneg")
                nc.vector.tensor_scalar(qneg, qm_i, -NEG, NEG, op0=ALU.mult, op1=ALU.add)
                # f32r matmul operands must be pre-rounded to f32r in SBUF
                CR = sb2.tile([D, LC], F32R, name="CR")
                for cj in range(NCJ):
                    eng = nc.gpsimd if cj % 2 == 0 else nc.vector
                    eng.tensor_copy(CR[:, cj * 512 : (cj + 1) * 512],
                                    C_sb[:, cj * 512 : (cj + 1) * 512])

                # c0[c] = sum_d C[d,c] w4C[d]; q1[q] = sum_d Q[d,q] w4Q[d]
                cq_p = ps_7.tile([128, NCT + NQT], F32, name="s7")
                for qi in range(NQT):
                    nc.tensor.matmul(cq_p[:, NCT + qi : NCT + qi + 1],
                                     Q_sb.bitcast(F32)[:, qi * 128 : (qi + 1) * 128],
                                     w4Q_sb, start=True, stop=True)
                for ci in range(NCT // 2):
                    nc.tensor.matmul(cq_p[:, ci : ci + 1],
                                     C_sb[:, ci * 128 : (ci + 1) * 128],
                                     w4C_sb, start=True, stop=True)
                bias_c = sb2.tile([128, NCT], F32, name="bias_c")
                nc.vector.tensor_tensor(bias_c[:, 0 : NCT // 2], cq_p[:, 0 : NCT // 2],
                                        cneg[:, 0 : NCT // 2], ALU.add)
                bias_q0 = sb2.tile([128, NQT], F32, name="bias_q0")
                nc.vector.tensor_tensor(bias_q0, cq_p[:, NCT : NCT + NQT], qneg, ALU.add)
                bias_q = sb2.tile([128, NQT], F32, name="bias_q")
                nc.vector.tensor_scalar_add(bias_q, bias_q0, bias_bc[:, 0:1])

                # ---- transposes: 4 per bank-wide group, in the s2pp bank
                #      (sequential lifetimes: transposes close before s2pp opens)
                Qt_sb = sb2.tile([128, NQT, 128], F32R, name="Qt_sb")
                tpq = ps_cs.tile([128, NQT, 128], F32R, name="cs")
                for qi in range(NQT):
                    nc.tensor.matmul(tpq[:, qi, :], Q_sb[:, qi * 128 : (qi + 1) * 128],
                                     identR, is_transpose=True,
                                     start=(qi == 0), stop=(qi == NQT - 1))
                nc.vector.tensor_copy(Qt_sb, tpq)
                Ct_sb = sb2.tile([128, NCT, 128], BF16, name="Ct_sb")

                def transpose_quad(g):
                    tp = ps_cs.tile([128, 4, 128], F32R, name="cs")
                    for k in range(4):
                        ci = 4 * g + k
                        nc.tensor.matmul(tp[:, k, :], CR[:, ci * 128 : (ci + 1) * 128],
                                         identR, is_transpose=True,
                                         start=(k == 0), stop=(k == 3))
                    if g % 2 == 0:
                        nc.vector.tensor_copy(Ct_sb[:, 4 * g : 4 * g + 4, :], tp)
                    else:
                        nc.scalar.copy(Ct_sb[:, 4 * g : 4 * g + 4, :], tp)

                transpose_quad(0)
                transpose_quad(1)
                transpose_quad(2)
                transpose_quad(3)

                # ---- E2 = exp(S + c-terms) in [c-part, q-free], fused with
                #      R[q,d] += E2[c,q]^T Ct[c,d] and s2sum[q] += colsum ----
                E2 = sb2.tile([128, NCT, LQ], BF16, name="E2")
                rp = ps_r.tile([128, NQT, 128], F32, name="rp")
                s2pp = ps_cs.tile([128, NQT], F32, name="cs")
                for ci in range(NCT):
                    sp = ps_7.tile([128, LQ], F32, name="s7")
                    nc.tensor.matmul(sp, CR[:, ci * 128 : (ci + 1) * 128], Qw,
                                     start=True, stop=True)
                    nc.scalar.activation(E2[:, ci, :], sp, AF.Exp,
                                         bias=bias_c[:, ci : ci + 1], scale=1.0)
                    for qi in range(NQT):
                        lhs = E2[:, ci, qi * 128 : (qi + 1) * 128]
                        first = ci == 0 and qi == 0
                        last = ci == NCT - 1 and qi == NQT - 1
                        nc.tensor.matmul(rp[:, qi, :], lhs, Ct_sb[:, ci, :],
                                         start=first, stop=last)
                        nc.tensor.matmul(s2pp[:, qi : qi + 1], lhs, ones_colB,
                                         start=first, stop=last)
                    if ci == 1:
                        # C-half-1-dependent front matter, emitted late so its
                        # waits don't head-of-line block the PE queue
                        for cil in range(NCT // 2, NCT):
                            nc.tensor.matmul(cq_p[:, cil : cil + 1],
                                             C_sb[:, cil * 128 : (cil + 1) * 128],
                                             w4C_sb, start=True, stop=True)
                        nc.vector.tensor_tensor(bias_c[:, NCT // 2 : NCT],
                                                cq_p[:, NCT // 2 : NCT],
                                                cneg[:, NCT // 2 : NCT], ALU.add)


            # ---- phase 8: E1T = exp(S^T + q-terms) in [q-part, c-free];
            #      per 512-col c-chunk: colsum -> broadcast rcp, A^T, B^T ----
            # R normalize sits here in program order so it doesn't head-of-line
            # block the DVE queue while the rp accumulation is still open.
            rs2 = sb2.tile([128, NQT], F32, name="rs2")
            nc.vector.reciprocal(rs2, s2pp)
            R_sb = sb2.tile([128, NQT, 128], F32R, name="R_sb")
            for qi in range(NQT):
                nc.vector.tensor_scalar_mul(R_sb[:, qi, :], rp[:, qi, :],
                                            rs2[:, qi : qi + 1])
            # C pass-through store: mid-kernel DMA lull, away from loads/tail
            nc.scalar.dma_start(out=out[b, 0:128, :], in_=C_sb)

            E1T = sb2.tile([128, NQT, LC], F32R, name="E1T")

            def consume(cj):
                sl = slice(cj * 512, (cj + 1) * 512)
                csp = ps_ab.tile([128, 512], F32, name="pab")
                for qi in range(NQT):
                    nc.tensor.matmul(csp, onesR128, E1T[:, qi, sl],
                                     start=(qi == 0), stop=(qi == NQT - 1))
                rcp_c = sb4.tile([128, 512], F32, name="rcp_c")
                nc.vector.reciprocal(rcp_c, csp)
                pa = ps_ab.tile([128, 512], F32, name="pab")
                for qi in range(NQT):
                    nc.tensor.matmul(pa, Qt_sb[:, qi, :], E1T[:, qi, sl],
                                     start=(qi == 0), stop=(qi == NQT - 1))
                pb = ps_ab.tile([128, 512], F32, name="pab")
                for qi in range(NQT):
                    nc.tensor.matmul(pb, R_sb[:, qi, :], E1T[:, qi, sl],
                                     start=(qi == 0), stop=(qi == NQT - 1))
                ACB = sb2.tile([128, 3, 512], F32, name="ACB")
                Bt_t = sb2.tile([128, 512], F32, name="Bt_t")
                nc.vector.tensor_tensor(ACB[:, 0, :], pa, rcp_c, ALU.mult)
                nc.vector.tensor_tensor(Bt_t, pb, rcp_c, ALU.mult)
                nc.vector.tensor_tensor(ACB[:, 1, :], C_sb[:, sl], ACB[:, 0, :],
                                        ALU.mult)
                nc.gpsimd.tensor_tensor(ACB[:, 2, :], C_sb[:, sl], Bt_t, ALU.mult)
                # one DMA stores [At|CA|CB] for this chunk: rows 128:512.
                # Last batch: split so the A rows ship before CA/CB finish.
                if b == BL - 1:
                    nc.sync.dma_start(out=out[b, 128:256, sl], in_=ACB[:, 0, :])
                    nc.sync.dma_start(out=out[b, 256:384, sl], in_=ACB[:, 1, :])
                    nc.sync.dma_start(out=out[b, 384:512, sl], in_=ACB[:, 2, :])
                else:
                    q = nc.sync if cj % 2 == 0 else nc.scalar
                    q.dma_start(
                        out=out[b, 128:512, sl].rearrange("(r p) c -> p r c", p=128),
                        in_=ACB,
                    )

            for cj in range(NCJ):
                sl = slice(cj * 512, (cj + 1) * 512)
                # first chunk's S/exp outranks the phase-7 tail so ACT rolls
                # straight from E2 into E1T exps at the phase boundary
                with (tc.high_priority(P8_BOOST) if cj == 0 else nullcontext()):
                    for qi in range(NQT):
                        sp = ps_8.tile([128, 512], F32, name="s8")
                        nc.tensor.matmul(sp, Qw[:, qi * 128 : (qi + 1) * 128],
                                         CR[:, sl], start=True, stop=True)
                        nc.scalar.activation(E1T[:, qi, sl], sp, AF.Exp,
                                             bias=bias_q[:, qi : qi + 1], scale=1.0)
                if cj > 0:
                    consume(cj - 1)
            consume(NCJ - 1)

    nc.finalize()
    return nc


_NC = None


def _get_nc():
    global _NC
    if _NC is None:
        _NC = _build_nc()
    return _NC


def kernel(C, Q, Cmask, Qmask, w4C, w4Q, w4mlu, bias, _trace=False):
    C = np.ascontiguousarray(np.asarray(C, dtype=np.float32))
    Q = np.ascontiguousarray(np.asarray(Q, dtype=np.float32))
    Cmask = np.ascontiguousarray(np.asarray(Cmask, dtype=np.int32))
    Qmask = np.ascontiguousarray(np.asarray(Qmask, dtype=np.int32))
    w4C = np.ascontiguousarray(np.asarray(w4C, dtype=np.float32))
    w4Q = np.ascontiguousarray(np.asarray(w4Q, dtype=np.float32))
    w4mlu = np.ascontiguousarray(np.asarray(w4mlu, dtype=np.float32))
    bias = np.ascontiguousarray(np.asarray(bias, dtype=np.float32))

    nc = _get_nc()
    in_maps = []
    for i in range(NCORES):
        s = slice(i * BL, (i + 1) * BL)
        in_maps.append({
            "C": C[s], "Q": Q[s], "Cmask": Cmask[s], "Qmask": Qmask[s],
            "w4C": w4C, "w4Q": w4Q, "w4mlu": w4mlu, "bias": bias,
        })
    res = run_bass_kernel_spmd(nc, in_maps, core_ids=list(range(NCORES)),
                               trace=_trace)
    out = np.concatenate([r["out"] for r in res.results], axis=0)
    if _trace:
        kernel._last_results = res
    return out


# revision 30
# speedup vs baseline: 1.1089x; 1.0216x over previous
"""CQAttention Trainium2 kernel.

Reference computation per batch b (C:[D,Lc], Q:[D,Lq], D=128, Lc=2048, Lq=512):
    Ct = C^T, Qt = Q^T
    S  = Ct@w4C + (Qt@w4Q)^T + (Ct*w4mlu)@Qt^T + bias        [Lc, Lq]
    S1 = softmax_q(S + NEG*(1-qmask))                         (over Lq)
    S2 = softmax_c(S + NEG*(1-cmask))                         (over Lc)
    A  = S1 @ Qt                                              [Lc, D]
    B  = S1 @ (S2^T @ Ct)     (right-assoc of (S1 S2^T) Ct)   [Lc, D]
    out= transpose(concat([Ct, A, Ct*A, Ct*B], -1))           [4D, Lc]

Kernel strategy (pure data parallel over batch: 16 batches / 8 cores):
  - S is never materialized: two matmul families compute S(sub2-part) in
    [c-part,q-free] and [q-part,c-free] layouts straight into PSUM; ScalarE
    exp() reads PSUM with a per-partition bias AP folding the softmax-relevant
    affine terms (terms constant along the softmax axis cancel). sub2 uses
    Qw = Q*w4mlu, so C needs no scaled copy (C^T diag(w) Q == C^T (w * Q)).
  - Tile-pipelined phases: each E2=[c,q] exp tile immediately feeds
    R += E2^T@Ct (whole-bank single PSUM accumulation group across all 4
    q-tiles x 16 c-tiles) plus a paired N=1 ones-matmul accumulating the S2
    normalizer in its own bank; PE(S), ACT(exp), PE(R) overlap per c-tile.
  - The S1 normalizer (colsum over q of E1T) is a ones[128,128]-weights
    matmul: every PSUM partition row carries the same colsum, so the
    reciprocal is directly partition-broadcast in SBUF - no DRAM bounce.
  - PSUM pools are phase-private (8 banks: 2 E2-S/transpose/cq, 2 E1T-S/csp,
    2 pa/pb, 1 rp, 1 s2sum) so batch b+1's front (Tile priority boost) never
    rotates behind batch b's tail allocations.
  - PE transposes run as 4-per-bank single accumulation groups (f32r identity,
    1.5 cyc/row) evacuated by one 512-row copy each.
"""

import numpy as np
from contextlib import ExitStack, nullcontext

import concourse.bass as bass
import concourse.mybir as mybir
import concourse.tile as tile
from concourse import bacc
from concourse.bass_utils import run_bass_kernel_spmd
from concourse.masks import make_identity

F32 = mybir.dt.float32
F32R = mybir.dt.float32r
I32 = mybir.dt.int32
AF = mybir.ActivationFunctionType
ALU = mybir.AluOpType
BF16 = mybir.dt.bfloat16

B, D, LC, LQ = 16, 128, 2048, 512
NCORES = 8
BL = B // NCORES          # batches per core
NEG = -1e30
NCT = LC // 128           # 16 c-tiles
NQT = LQ // 128           # 4 q-tiles
NCJ = LC // 512           # 4 c-chunks (free-dim)
HIPRI_OFF = 205
P8_BOOST = 40


def _build_nc():
    nc = bacc.Bacc("TRN2", target_bir_lowering=False)
    Ci = nc.dram_tensor("C", [BL, D, LC], F32, kind="ExternalInput")
    Qi = nc.dram_tensor("Q", [BL, D, LQ], F32, kind="ExternalInput")
    CMi = nc.dram_tensor("Cmask", [BL, LC], I32, kind="ExternalInput")
    QMi = nc.dram_tensor("Qmask", [BL, LQ], I32, kind="ExternalInput")
    w4C = nc.dram_tensor("w4C", [D, 1], F32, kind="ExternalInput")
    w4Q = nc.dram_tensor("w4Q", [D, 1], F32, kind="ExternalInput")
    w4mlu = nc.dram_tensor("w4mlu", [1, 1, D], F32, kind="ExternalInput")
    biasi = nc.dram_tensor("bias", [1], F32, kind="ExternalInput")
    out = nc.dram_tensor("out", [BL, 4 * D, LC], F32, kind="ExternalOutput")

    with tile.TileContext(nc) as tc, ExitStack() as ctx:
        const = ctx.enter_context(tc.tile_pool(name="const", bufs=1))
        sb2 = ctx.enter_context(tc.tile_pool(name="sb2", bufs=2))
        sb4 = ctx.enter_context(tc.tile_pool(name="sb4", bufs=4))
        # PSUM: 8 banks, phase-private pools (see module docstring)
        ps_7 = ctx.enter_context(tc.tile_pool(name="ps_7", bufs=2, space="PSUM"))
        ps_8 = ctx.enter_context(tc.tile_pool(name="ps_8", bufs=2, space="PSUM"))
        ps_ab = ctx.enter_context(tc.tile_pool(name="ps_ab", bufs=2, space="PSUM"))
        ps_r = ctx.enter_context(tc.tile_pool(name="ps_r", bufs=1, space="PSUM"))
        ps_cs = ctx.enter_context(tc.tile_pool(name="ps_cs", bufs=1, space="PSUM"))

        # ---- constants; identity first (Pool), then SWDGE w-consts so the
        #      weight loads don't sit behind identity building on Pool ----
        ident0 = const.tile([D, D], F32, name="ident0")
        make_identity(nc, ident0)
        identR = const.tile([D, D], F32R, name="identR")
        nc.vector.tensor_copy(identR, ident0)
        ones_f = const.tile([D, D], F32, name="ones_f")
        nc.vector.memset(ones_f, 1.0)
        onesR128 = const.tile([D, D], F32R, name="onesR128")
        nc.vector.tensor_copy(onesR128, ones_f)
        ones_colB = const.tile([D, 1], BF16, name="ones_colB")
        nc.vector.tensor_copy(ones_colB, ones_f[:, 0:1])
        wmlu_sb = const.tile([D, 1], F32, name="wmlu_sb")
        nc.gpsimd.dma_start(out=wmlu_sb, in_=w4mlu.ap().rearrange("a b d -> d (a b)"))
        w4C_sb = const.tile([D, 1], F32, name="w4C_sb")
        nc.gpsimd.dma_start(out=w4C_sb, in_=w4C[:, :])
        w4Q_sb = const.tile([D, 1], F32, name="w4Q_sb")
        nc.gpsimd.dma_start(out=w4Q_sb, in_=w4Q[:, :])
        bias_bc = const.tile([D, 1], F32, name="bias_bc")
        nc.scalar.dma_start(out=bias_bc, in_=biasi.ap().partition_broadcast(D))

        for b in range(BL):
            with (tc.high_priority(HIPRI_OFF) if b > 0 else nullcontext()):
                # ---- loads: Q, C-half0, masks, C-half1 ----
                Q_sb = sb2.tile([D, LQ], F32R, name="Q_sb")
                nc.sync.dma_start(out=Q_sb, in_=Qi[b, :, :].bitcast(F32R))
                C_sb = sb2.tile([D, LC], F32, name="C_sb")
                nc.sync.dma_start(out=C_sb[:, 0:1024], in_=Ci[b, :, 0:1024])
                cm_i = sb2.tile([128, NCT], I32, name="cm_i")
                nc.sync.dma_start(out=cm_i, in_=CMi[b, :].rearrange("(i p) -> p i", p=128))
                qm_i = sb2.tile([128, NQT], I32, name="qm_i")
                nc.sync.dma_start(out=qm_i, in_=QMi[b, :].rearrange("(i p) -> p i", p=128))
                nc.sync.dma_start(out=C_sb[:, 1024:2048], in_=Ci[b, :, 1024:2048])

                # ---- small prep ----
                Qw = sb2.tile([D, LQ], F32R, name="Qw")
                nc.vector.tensor_scalar_mul(Qw, Q_sb, wmlu_sb[:, 0:1])
                # mask -> NEG*(1-m):  m*(-NEG) + NEG
                cneg = sb2.tile([128, NCT], F32, name="cneg")
                nc.vector.tensor_scalar(cneg, cm_i, -NEG, NEG, op0=ALU.mult, op1=ALU.add)
                qneg = sb2.tile([128, NQT], F32, name="qneg")
                nc.vector.tensor_scalar(qneg, qm_i, -NEG, NEG, op0=ALU.mult, op1=ALU.add)
                # f32r matmul operands must be pre-rounded to f32r in SBUF
                CR = sb2.tile([D, LC], F32R, name="CR")
                for cj in range(NCJ):
                    eng = nc.gpsimd if cj % 2 == 0 else nc.vector
                    eng.tensor_copy(CR[:, cj * 512 : (cj + 1) * 512],
                                    C_sb[:, cj * 512 : (cj + 1) * 512])

                # c0[c] = sum_d C[d,c] w4C[d]; q1[q] = sum_d Q[d,q] w4Q[d]
                cq_p = ps_7.tile([128, NCT + NQT], F32, name="s7")
                for qi in range(NQT):
                    nc.tensor.matmul(cq_p[:, NCT + qi : NCT + qi + 1],
                                     Q_sb.bitcast(F32)[:, qi * 128 : (qi + 1) * 128],
                                     w4Q_sb, start=True, stop=True)
                for ci in range(NCT // 2):
                    nc.tensor.matmul(cq_p[:, ci : ci + 1],
                                     C_sb[:, ci * 128 : (ci + 1) * 128],
                                     w4C_sb, start=True, stop=True)
                bias_c = sb2.tile([128, NCT], F32, name="bias_c")
                nc.vector.tensor_tensor(bias_c[:, 0 : NCT // 2], cq_p[:, 0 : NCT // 2],
                                        cneg[:, 0 : NCT // 2], ALU.add)
                bias_q0 = sb2.tile([128, NQT], F32, name="bias_q0")
                nc.vector.tensor_tensor(bias_q0, cq_p[:, NCT : NCT + NQT], qneg, ALU.add)
                bias_q = sb2.tile([128, NQT], F32, name="bias_q")
                nc.vector.tensor_scalar_add(bias_q, bias_q0, bias_bc[:, 0:1])

                # ---- transposes: 4 per bank-wide group, in the s2pp bank
                #      (sequential lifetimes: transposes close before s2pp opens)
                Qt_sb = sb2.tile([128, NQT, 128], F32R, name="Qt_sb")
                tpq = ps_cs.tile([128, NQT, 128], F32R, name="cs")
                for qi in range(NQT):
                    nc.tensor.matmul(tpq[:, qi, :], Q_sb[:, qi * 128 : (qi + 1) * 128],
                                     identR, is_transpose=True,
                                     start=(qi == 0), stop=(qi == NQT - 1))
                nc.vector.tensor_copy(Qt_sb, tpq)
                Ct_sb = sb2.tile([128, NCT, 128], BF16, name="Ct_sb")

                def transpose_quad(g):
                    tp = ps_cs.tile([128, 4, 128], F32R, name="cs")
                    for k in range(4):
                        ci = 4 * g + k
                        nc.tensor.matmul(tp[:, k, :], CR[:, ci * 128 : (ci + 1) * 128],
                                         identR, is_transpose=True,
                                         start=(k == 0), stop=(k == 3))
                    if g % 2 == 0:
                        nc.vector.tensor_copy(Ct_sb[:, 4 * g : 4 * g + 4, :], tp)
                    else:
                        nc.scalar.copy(Ct_sb[:, 4 * g : 4 * g + 4, :], tp)

                transpose_quad(0)
                transpose_quad(1)
                transpose_quad(2)
                transpose_quad(3)

                # ---- E2 = exp(S + c-terms) in [c-part, q-free], fused with
                #      R[q,d] += E2[c,q]^T Ct[c,d] and s2sum[q] += colsum ----
                E2 = sb2.tile([128, NCT, LQ], BF16, name="E2")
                rp = ps_r.tile([128, NQT, 128], F32, name="rp")
                s2pp = ps_cs.tile([128, NQT], F32, name="cs")
                for ci in range(NCT):
                    sp = ps_7.tile([128, LQ], F32, name="s7")
                    nc.tensor.matmul(sp, CR[:, ci * 128 : (ci + 1) * 128], Qw,
                                     start=True, stop=True)
                    nc.scalar.activation(E2[:, ci, :], sp, AF.Exp,
                                         bias=bias_c[:, ci : ci + 1], scale=1.0)
                    for qi in range(NQT):
                        lhs = E2[:, ci, qi * 128 : (qi + 1) * 128]
                        first = ci == 0 and qi == 0
                        last = ci == NCT - 1 and qi == NQT - 1
                        nc.tensor.matmul(rp[:, qi, :], lhs, Ct_sb[:, ci, :],
                                         start=first, stop=last)
                        nc.tensor.matmul(s2pp[:, qi : qi + 1], lhs, ones_colB,
                                         start=first, stop=last)
                    if ci == 1:
                        # C-half-1-dependent front matter, emitted late so its
                        # waits don't head-of-line block the PE queue
                        for cil in range(NCT // 2, NCT):
                            nc.tensor.matmul(cq_p[:, cil : cil + 1],
                                             C_sb[:, cil * 128 : (cil + 1) * 128],
                                             w4C_sb, start=True, stop=True)
                        nc.vector.tensor_tensor(bias_c[:, NCT // 2 : NCT],
                                                cq_p[:, NCT // 2 : NCT],
                                                cneg[:, NCT // 2 : NCT], ALU.add)


            # ---- phase 8: E1T = exp(S^T + q-terms) in [q-part, c-free];
            #      per 512-col c-chunk: colsum -> broadcast rcp, A^T, B^T ----
            # R normalize sits here in program order so it doesn't head-of-line
            # block the DVE queue while the rp accumulation is still open.
            rs2 = sb2.tile([128, NQT], F32, name="rs2")
            nc.vector.reciprocal(rs2, s2pp)
            R_sb = sb2.tile([128, NQT, 128], F32R, name="R_sb")
            for qi in range(NQT):
                nc.vector.tensor_scalar_mul(R_sb[:, qi, :], rp[:, qi, :],
                                            rs2[:, qi : qi + 1])
            # C pass-through store: mid-kernel DMA lull, away from loads/tail
            nc.scalar.dma_start(out=out[b, 0:128, :], in_=C_sb)

            E1T = sb2.tile([128, NQT, LC], F32R, name="E1T")

            def consume(cj):
                sl = slice(cj * 512, (cj + 1) * 512)
                csp = ps_ab.tile([128, 512], F32, name="pab")
                for qi in range(NQT):
                    nc.tensor.matmul(csp, onesR128, E1T[:, qi, sl],
                                     start=(qi == 0), stop=(qi == NQT - 1))
                rcp_c = sb4.tile([128, 512], F32, name="rcp_c")
                nc.vector.reciprocal(rcp_c, csp)
                pa = ps_ab.tile([128, 512], F32, name="pab")
                for qi in range(NQT):
                    nc.tensor.matmul(pa, Qt_sb[:, qi, :], E1T[:, qi, sl],
                                     start=(qi == 0), stop=(qi == NQT - 1))
                pb = ps_ab.tile([128, 512], F32, name="pab")
                for qi in range(NQT):
                    nc.tensor.matmul(pb, R_sb[:, qi, :], E1T[:, qi, sl],
                                     start=(qi == 0), stop=(qi == NQT - 1))
                ACB = sb2.tile([128, 3, 512], F32, name="ACB")
                Bt_t = sb2.tile([128, 512], F32, name="Bt_t")
                nc.vector.tensor_tensor(ACB[:, 0, :], pa, rcp_c, ALU.mult)
                nc.vector.tensor_tensor(Bt_t, pb, rcp_c, ALU.mult)
                nc.vector.tensor_tensor(ACB[:, 1, :], C_sb[:, sl], ACB[:, 0, :],
                                        ALU.mult)
                nc.gpsimd.tensor_tensor(ACB[:, 2, :], C_sb[:, sl], Bt_t, ALU.mult)
                # one DMA stores [At|CA|CB] for this chunk: rows 128:512.
                # Last batch: split so the A rows ship before CA/CB finish.
                if b == BL - 1:
                    nc.sync.dma_start(out=out[b, 128:256, sl], in_=ACB[:, 0, :])
                    nc.sync.dma_start(out=out[b, 256:384, sl], in_=ACB[:, 1, :])
                    nc.sync.dma_start(out=out[b, 384:512, sl], in_=ACB[:, 2, :])
                else:
                    q = nc.sync if cj % 2 == 0 else nc.scalar
                    q.dma_start(
                        out=out[b, 128:512, sl].rearrange("(r p) c -> p r c", p=128),
                        in_=ACB,
                    )

            for cj in range(NCJ):
                sl = slice(cj * 512, (cj + 1) * 512)
                # first chunk's S/exp outranks the phase-7 tail so ACT rolls
                # straight from E2 into E1T exps at the phase boundary
                with (tc.high_priority(P8_BOOST) if cj == 0 else nullcontext()):
                    for qi in range(NQT):
                        sp = ps_8.tile([128, 512], F32, name="s8")
                        nc.tensor.matmul(sp, Qw[:, qi * 128 : (qi + 1) * 128],
                                         CR[:, sl], start=True, stop=True)
                        nc.scalar.activation(E1T[:, qi, sl], sp, AF.Exp,
                                             bias=bias_q[:, qi : qi + 1], scale=1.0)
                if cj > 0:
                    consume(cj - 1)
            consume(NCJ - 1)

    nc.finalize()
    return nc


_NC = None


def _get_nc():
    global _NC
    if _NC is None:
        _NC = _build_nc()
    return _NC


def kernel(C, Q, Cmask, Qmask, w4C, w4Q, w4mlu, bias, _trace=False):
    C = np.ascontiguousarray(np.asarray(C, dtype=np.float32))
    Q = np.ascontiguousarray(np.asarray(Q, dtype=np.float32))
    Cmask = np.ascontiguousarray(np.asarray(Cmask, dtype=np.int32))
    Qmask = np.ascontiguousarray(np.asarray(Qmask, dtype=np.int32))
    w4C = np.ascontiguousarray(np.asarray(w4C, dtype=np.float32))
    w4Q = np.ascontiguousarray(np.asarray(w4Q, dtype=np.float32))
    w4mlu = np.ascontiguousarray(np.asarray(w4mlu, dtype=np.float32))
    bias = np.ascontiguousarray(np.asarray(bias, dtype=np.float32))

    nc = _get_nc()
    in_maps = []
    for i in range(NCORES):
        s = slice(i * BL, (i + 1) * BL)
        in_maps.append({
            "C": C[s], "Q": Q[s], "Cmask": Cmask[s], "Qmask": Qmask[s],
            "w4C": w4C, "w4Q": w4Q, "w4mlu": w4mlu, "bias": bias,
        })
    res = run_bass_kernel_spmd(nc, in_maps, core_ids=list(range(NCORES)),
                               trace=_trace)
    out = np.concatenate([r["out"] for r in res.results], axis=0)
    if _trace:
        kernel._last_results = res
    return out


# revision 34
# speedup vs baseline: 1.1242x; 1.0138x over previous
"""CQAttention Trainium2 kernel.

Reference computation per batch b (C:[D,Lc], Q:[D,Lq], D=128, Lc=2048, Lq=512):
    Ct = C^T, Qt = Q^T
    S  = Ct@w4C + (Qt@w4Q)^T + (Ct*w4mlu)@Qt^T + bias        [Lc, Lq]
    S1 = softmax_q(S + NEG*(1-qmask))                         (over Lq)
    S2 = softmax_c(S + NEG*(1-cmask))                         (over Lc)
    A  = S1 @ Qt                                              [Lc, D]
    B  = S1 @ (S2^T @ Ct)     (right-assoc of (S1 S2^T) Ct)   [Lc, D]
    out= transpose(concat([Ct, A, Ct*A, Ct*B], -1))           [4D, Lc]

Kernel strategy (pure data parallel over batch: 16 batches / 8 cores):
  - S is never materialized: two matmul families compute S(sub2-part) in
    [c-part,q-free] and [q-part,c-free] layouts straight into PSUM; ScalarE
    exp() reads PSUM with a per-partition bias AP folding the softmax-relevant
    affine terms (terms constant along the softmax axis cancel). sub2 uses
    Qw = Q*w4mlu, so C needs no scaled copy (C^T diag(w) Q == C^T (w * Q)).
  - Tile-pipelined phases: each E2=[c,q] exp tile immediately feeds
    R += E2^T@Ct (whole-bank single PSUM accumulation group across all 4
    q-tiles x 16 c-tiles) plus a paired N=1 ones-matmul accumulating the S2
    normalizer in its own bank; PE(S), ACT(exp), PE(R) overlap per c-tile.
  - The S1 normalizer (colsum over q of E1T) is a ones[128,128]-weights
    matmul: every PSUM partition row carries the same colsum, so the
    reciprocal is directly partition-broadcast in SBUF - no DRAM bounce.
  - PSUM pools are phase-private (8 banks: 2 E2-S/transpose/cq, 2 E1T-S/csp,
    2 pa/pb, 1 rp, 1 s2sum) so batch b+1's front (Tile priority boost) never
    rotates behind batch b's tail allocations.
  - PE transposes run as 4-per-bank single accumulation groups (f32r identity,
    1.5 cyc/row) evacuated by one 512-row copy each.
"""

import numpy as np
from contextlib import ExitStack, nullcontext

import concourse.bass as bass
import concourse.mybir as mybir
import concourse.tile as tile
from concourse import bacc
from concourse.bass_utils import run_bass_kernel_spmd
from concourse.masks import make_identity

F32 = mybir.dt.float32
F32R = mybir.dt.float32r
I32 = mybir.dt.int32
AF = mybir.ActivationFunctionType
ALU = mybir.AluOpType
BF16 = mybir.dt.bfloat16

B, D, LC, LQ = 16, 128, 2048, 512
NCORES = 8
BL = B // NCORES          # batches per core
NEG = -1e30
NCT = LC // 128           # 16 c-tiles
NQT = LQ // 128           # 4 q-tiles
NCJ = LC // 512           # 4 c-chunks (free-dim)
HIPRI_OFF = 205
P8_BOOST = 40


def _build_nc():
    nc = bacc.Bacc("TRN2", target_bir_lowering=False)
    Ci = nc.dram_tensor("C", [BL, D, LC], F32, kind="ExternalInput")
    Qi = nc.dram_tensor("Q", [BL, D, LQ], F32, kind="ExternalInput")
    CMi = nc.dram_tensor("Cmask", [BL, LC], I32, kind="ExternalInput")
    QMi = nc.dram_tensor("Qmask", [BL, LQ], I32, kind="ExternalInput")
    w4C = nc.dram_tensor("w4C", [D, 1], F32, kind="ExternalInput")
    w4Q = nc.dram_tensor("w4Q", [D, 1], F32, kind="ExternalInput")
    w4mlu = nc.dram_tensor("w4mlu", [1, 1, D], F32, kind="ExternalInput")
    biasi = nc.dram_tensor("bias", [1], F32, kind="ExternalInput")
    out = nc.dram_tensor("out", [BL, 4 * D, LC], F32, kind="ExternalOutput")

    with tile.TileContext(nc) as tc, ExitStack() as ctx:
        const = ctx.enter_context(tc.tile_pool(name="const", bufs=1))
        sb2 = ctx.enter_context(tc.tile_pool(name="sb2", bufs=2))
        sb4 = ctx.enter_context(tc.tile_pool(name="sb4", bufs=4))
        # PSUM: 8 banks, phase-private pools (see module docstring)
        ps_7 = ctx.enter_context(tc.tile_pool(name="ps_7", bufs=2, space="PSUM"))
        ps_8 = ctx.enter_context(tc.tile_pool(name="ps_8", bufs=2, space="PSUM"))
        ps_ab = ctx.enter_context(tc.tile_pool(name="ps_ab", bufs=2, space="PSUM"))
        ps_r = ctx.enter_context(tc.tile_pool(name="ps_r", bufs=1, space="PSUM"))
        ps_cs = ctx.enter_context(tc.tile_pool(name="ps_cs", bufs=1, space="PSUM"))

        # ---- constants; identity first (Pool), then SWDGE w-consts so the
        #      weight loads don't sit behind identity building on Pool ----
        ident0 = const.tile([D, D], F32, name="ident0")
        make_identity(nc, ident0)
        identR = const.tile([D, D], F32R, name="identR")
        nc.vector.tensor_copy(identR, ident0)
        ones_f = const.tile([D, D], F32, name="ones_f")
        nc.vector.memset(ones_f, 1.0)
        onesR128 = const.tile([D, D], F32R, name="onesR128")
        nc.vector.tensor_copy(onesR128, ones_f)
        ones_colB = const.tile([D, 1], BF16, name="ones_colB")
        nc.vector.tensor_copy(ones_colB, ones_f[:, 0:1])
        wmlu_sb = const.tile([D, 1], F32, name="wmlu_sb")
        nc.gpsimd.dma_start(out=wmlu_sb, in_=w4mlu.ap().rearrange("a b d -> d (a b)"))
        w4C_sb = const.tile([D, 1], F32, name="w4C_sb")
        nc.gpsimd.dma_start(out=w4C_sb, in_=w4C[:, :])
        w4Q_sb = const.tile([D, 1], F32, name="w4Q_sb")
        nc.gpsimd.dma_start(out=w4Q_sb, in_=w4Q[:, :])
        bias_bc = const.tile([D, 1], F32, name="bias_bc")
        nc.scalar.dma_start(out=bias_bc, in_=biasi.ap().partition_broadcast(D))

        for b in range(BL):
            with (tc.high_priority(HIPRI_OFF) if b > 0 else nullcontext()):
                # ---- loads: Q, C-half0, masks, C-half1 ----
                Q_sb = sb2.tile([D, LQ], F32R, name="Q_sb")
                nc.sync.dma_start(out=Q_sb, in_=Qi[b, :, :].bitcast(F32R))
                C_sb = sb2.tile([D, LC], F32, name="C_sb")
                nc.sync.dma_start(out=C_sb[:, 0:1024], in_=Ci[b, :, 0:1024])
                cm_i = sb2.tile([128, NCT], I32, name="cm_i")
                nc.sync.dma_start(out=cm_i, in_=CMi[b, :].rearrange("(i p) -> p i", p=128))
                qm_i = sb2.tile([128, NQT], I32, name="qm_i")
                nc.sync.dma_start(out=qm_i, in_=QMi[b, :].rearrange("(i p) -> p i", p=128))
                nc.sync.dma_start(out=C_sb[:, 1024:2048], in_=Ci[b, :, 1024:2048])

                # ---- small prep ----
                Qw = sb2.tile([D, LQ], F32R, name="Qw")
                nc.vector.tensor_scalar_mul(Qw, Q_sb, wmlu_sb[:, 0:1])
                # mask -> NEG*(1-m):  m*(-NEG) + NEG
                cneg = sb2.tile([128, NCT], F32, name="cneg")
                nc.vector.tensor_scalar(cneg, cm_i, -NEG, NEG, op0=ALU.mult, op1=ALU.add)
                qneg = sb2.tile([128, NQT], F32, name="qneg")
                nc.vector.tensor_scalar(qneg, qm_i, -NEG, NEG, op0=ALU.mult, op1=ALU.add)
                # f32r matmul operands must be pre-rounded to f32r in SBUF
                CR = sb2.tile([D, LC], F32R, name="CR")
                for cj in range(NCJ):
                    eng = nc.gpsimd if cj % 2 == 0 else nc.vector
                    eng.tensor_copy(CR[:, cj * 512 : (cj + 1) * 512],
                                    C_sb[:, cj * 512 : (cj + 1) * 512])

                # ---- Q-only transposes first (C loads still in flight) ----
                Qt_sb = sb2.tile([128, NQT, 128], F32R, name="Qt_sb")
                tpq = ps_cs.tile([128, NQT, 128], F32R, name="cs")
                for qi in range(NQT):
                    nc.tensor.matmul(tpq[:, qi, :], Q_sb[:, qi * 128 : (qi + 1) * 128],
                                     identR, is_transpose=True,
                                     start=(qi == 0), stop=(qi == NQT - 1))
                nc.vector.tensor_copy(Qt_sb, tpq)

                # c0[c] = sum_d C[d,c] w4C[d]; q1[q] = sum_d Q[d,q] w4Q[d]
                cq_p = ps_7.tile([128, NCT + NQT], F32, name="s7")
                for qi in range(NQT):
                    nc.tensor.matmul(cq_p[:, NCT + qi : NCT + qi + 1],
                                     Q_sb.bitcast(F32)[:, qi * 128 : (qi + 1) * 128],
                                     w4Q_sb, start=True, stop=True)
                for ci in range(NCT // 2):
                    nc.tensor.matmul(cq_p[:, ci : ci + 1],
                                     C_sb[:, ci * 128 : (ci + 1) * 128],
                                     w4C_sb, start=True, stop=True)
                bias_c = sb2.tile([128, NCT], F32, name="bias_c")
                nc.vector.tensor_tensor(bias_c[:, 0 : NCT // 2], cq_p[:, 0 : NCT // 2],
                                        cneg[:, 0 : NCT // 2], ALU.add)
                bias_q0 = sb2.tile([128, NQT], F32, name="bias_q0")
                nc.vector.tensor_tensor(bias_q0, cq_p[:, NCT : NCT + NQT], qneg, ALU.add)
                bias_q = sb2.tile([128, NQT], F32, name="bias_q")
                nc.vector.tensor_scalar_add(bias_q, bias_q0, bias_bc[:, 0:1])

                Ct_sb = sb2.tile([128, NCT, 128], BF16, name="Ct_sb")

                def transpose_quad(g):
                    tp = ps_cs.tile([128, 4, 128], F32R, name="cs")
                    for k in range(4):
                        ci = 4 * g + k
                        nc.tensor.matmul(tp[:, k, :], CR[:, ci * 128 : (ci + 1) * 128],
                                         identR, is_transpose=True,
                                         start=(k == 0), stop=(k == 3))
                    if g % 2 == 0:
                        nc.vector.tensor_copy(Ct_sb[:, 4 * g : 4 * g + 4, :], tp)
                    else:
                        nc.scalar.copy(Ct_sb[:, 4 * g : 4 * g + 4, :], tp)

                transpose_quad(0)
                transpose_quad(1)
                transpose_quad(2)
                transpose_quad(3)

                # ---- E2 = exp(S + c-terms) in [c-part, q-free], fused with
                #      R[q,d] += E2[c,q]^T Ct[c,d] and s2sum[q] += colsum ----
                E2 = sb2.tile([128, NCT, LQ], BF16, name="E2")
                rp = ps_r.tile([128, NQT, 128], F32, name="rp")
                s2pp = ps_cs.tile([128, NQT], F32, name="cs")
                for ci in range(NCT):
                    sp = ps_7.tile([128, LQ], F32, name="s7")
                    nc.tensor.matmul(sp, CR[:, ci * 128 : (ci + 1) * 128], Qw,
                                     start=True, stop=True)
                    nc.scalar.activation(E2[:, ci, :], sp, AF.Exp,
                                         bias=bias_c[:, ci : ci + 1], scale=1.0)
                    for qi in range(NQT):
                        lhs = E2[:, ci, qi * 128 : (qi + 1) * 128]
                        first = ci == 0 and qi == 0
                        last = ci == NCT - 1 and qi == NQT - 1
                        nc.tensor.matmul(rp[:, qi, :], lhs, Ct_sb[:, ci, :],
                                         start=first, stop=last)
                        nc.tensor.matmul(s2pp[:, qi : qi + 1], lhs, ones_colB,
                                         start=first, stop=last)
                    if ci == 1:
                        # C-half-1-dependent front matter, emitted late so its
                        # waits don't head-of-line block the PE queue
                        for cil in range(NCT // 2, NCT):
                            nc.tensor.matmul(cq_p[:, cil : cil + 1],
                                             C_sb[:, cil * 128 : (cil + 1) * 128],
                                             w4C_sb, start=True, stop=True)
                        nc.vector.tensor_tensor(bias_c[:, NCT // 2 : NCT],
                                                cq_p[:, NCT // 2 : NCT],
                                                cneg[:, NCT // 2 : NCT], ALU.add)


            # ---- phase 8: E1T = exp(S^T + q-terms) in [q-part, c-free];
            #      per 512-col c-chunk: colsum -> broadcast rcp, A^T, B^T ----
            # R normalize sits here in program order so it doesn't head-of-line
            # block the DVE queue while the rp accumulation is still open.
            rs2 = sb2.tile([128, NQT], F32, name="rs2")
            nc.vector.reciprocal(rs2, s2pp)
            R_sb = sb2.tile([128, NQT, 128], F32R, name="R_sb")
            for qi in range(NQT):
                nc.vector.tensor_scalar_mul(R_sb[:, qi, :], rp[:, qi, :],
                                            rs2[:, qi : qi + 1])
            # C pass-through store: mid-kernel DMA lull, away from loads/tail
            nc.scalar.dma_start(out=out[b, 0:128, :], in_=C_sb)

            E1T = sb2.tile([128, NQT, LC], F32R, name="E1T")

            def consume(cj):
                sl = slice(cj * 512, (cj + 1) * 512)
                csp = ps_ab.tile([128, 512], F32, name="pab")
                for qi in range(NQT):
                    nc.tensor.matmul(csp, onesR128, E1T[:, qi, sl],
                                     start=(qi == 0), stop=(qi == NQT - 1))
                rcp_c = sb4.tile([128, 512], F32, name="rcp_c")
                nc.vector.reciprocal(rcp_c, csp)
                pa = ps_ab.tile([128, 512], F32, name="pab")
                for qi in range(NQT):
                    nc.tensor.matmul(pa, Qt_sb[:, qi, :], E1T[:, qi, sl],
                                     start=(qi == 0), stop=(qi == NQT - 1))
                pb = ps_ab.tile([128, 512], F32, name="pab")
                for qi in range(NQT):
                    nc.tensor.matmul(pb, R_sb[:, qi, :], E1T[:, qi, sl],
                                     start=(qi == 0), stop=(qi == NQT - 1))
                ACB = sb2.tile([128, 3, 512], F32, name="ACB")
                Bt_t = sb2.tile([128, 512], F32, name="Bt_t")
                nc.vector.tensor_tensor(ACB[:, 0, :], pa, rcp_c, ALU.mult)
                nc.vector.tensor_tensor(Bt_t, pb, rcp_c, ALU.mult)
                e1, e2 = (nc.vector, nc.gpsimd) if cj % 2 == 0 else (nc.gpsimd, nc.vector)
                e1.tensor_tensor(ACB[:, 1, :], C_sb[:, sl], ACB[:, 0, :], ALU.mult)
                e2.tensor_tensor(ACB[:, 2, :], C_sb[:, sl], Bt_t, ALU.mult)
                # one DMA stores [At|CA|CB] for this chunk: rows 128:512.
                # Last batch: split so the A rows ship before CA/CB finish.
                if b == BL - 1:
                    nc.sync.dma_start(out=out[b, 128:256, sl], in_=ACB[:, 0, :])
                    nc.sync.dma_start(out=out[b, 256:384, sl], in_=ACB[:, 1, :])
                    nc.sync.dma_start(out=out[b, 384:512, sl], in_=ACB[:, 2, :])
                else:
                    q = nc.sync if cj % 2 == 0 else nc.scalar
                    q.dma_start(
                        out=out[b, 128:512, sl].rearrange("(r p) c -> p r c", p=128),
                        in_=ACB,
                    )

            for cj in range(NCJ):
                sl = slice(cj * 512, (cj + 1) * 512)
                # first chunk's S/exp outranks the phase-7 tail so ACT rolls
                # straight from E2 into E1T exps at the phase boundary
                with (tc.high_priority(P8_BOOST) if cj == 0 else nullcontext()):
                    for qi in range(NQT):
                        sp = ps_8.tile([128, 512], F32, name="s8")
                        nc.tensor.matmul(sp, Qw[:, qi * 128 : (qi + 1) * 128],
                                         CR[:, sl], start=True, stop=True)
                        nc.scalar.activation(E1T[:, qi, sl], sp, AF.Exp,
                                             bias=bias_q[:, qi : qi + 1], scale=1.0)
                if cj > 0:
                    consume(cj - 1)
            consume(NCJ - 1)

    nc.finalize()
    return nc


_NC = None


def _get_nc():
    global _NC
    if _NC is None:
        _NC = _build_nc()
    return _NC


def kernel(C, Q, Cmask, Qmask, w4C, w4Q, w4mlu, bias, _trace=False):
    C = np.ascontiguousarray(np.asarray(C, dtype=np.float32))
    Q = np.ascontiguousarray(np.asarray(Q, dtype=np.float32))
    Cmask = np.ascontiguousarray(np.asarray(Cmask, dtype=np.int32))
    Qmask = np.ascontiguousarray(np.asarray(Qmask, dtype=np.int32))
    w4C = np.ascontiguousarray(np.asarray(w4C, dtype=np.float32))
    w4Q = np.ascontiguousarray(np.asarray(w4Q, dtype=np.float32))
    w4mlu = np.ascontiguousarray(np.asarray(w4mlu, dtype=np.float32))
    bias = np.ascontiguousarray(np.asarray(bias, dtype=np.float32))

    nc = _get_nc()
    in_maps = []
    for i in range(NCORES):
        s = slice(i * BL, (i + 1) * BL)
        in_maps.append({
            "C": C[s], "Q": Q[s], "Cmask": Cmask[s], "Qmask": Qmask[s],
            "w4C": w4C, "w4Q": w4Q, "w4mlu": w4mlu, "bias": bias,
        })
    res = run_bass_kernel_spmd(nc, in_maps, core_ids=list(range(NCORES)),
                               trace=_trace)
    out = np.concatenate([r["out"] for r in res.results], axis=0)
    if _trace:
        kernel._last_results = res
    return out


# revision 36
# speedup vs baseline: 1.1326x; 1.0075x over previous
"""CQAttention Trainium2 kernel.

Reference computation per batch b (C:[D,Lc], Q:[D,Lq], D=128, Lc=2048, Lq=512):
    Ct = C^T, Qt = Q^T
    S  = Ct@w4C + (Qt@w4Q)^T + (Ct*w4mlu)@Qt^T + bias        [Lc, Lq]
    S1 = softmax_q(S + NEG*(1-qmask))                         (over Lq)
    S2 = softmax_c(S + NEG*(1-cmask))                         (over Lc)
    A  = S1 @ Qt                                              [Lc, D]
    B  = S1 @ (S2^T @ Ct)     (right-assoc of (S1 S2^T) Ct)   [Lc, D]
    out= transpose(concat([Ct, A, Ct*A, Ct*B], -1))           [4D, Lc]

Kernel strategy (pure data parallel over batch: 16 batches / 8 cores):
  - S is never materialized: two matmul families compute S(sub2-part) in
    [c-part,q-free] and [q-part,c-free] layouts straight into PSUM; ScalarE
    exp() reads PSUM with a per-partition bias AP folding the softmax-relevant
    affine terms (terms constant along the softmax axis cancel). sub2 uses
    Qw = Q*w4mlu, so C needs no scaled copy (C^T diag(w) Q == C^T (w * Q)).
  - Tile-pipelined phases: each E2=[c,q] exp tile immediately feeds
    R += E2^T@Ct (whole-bank single PSUM accumulation group across all 4
    q-tiles x 16 c-tiles) plus a paired N=1 ones-matmul accumulating the S2
    normalizer in its own bank; PE(S), ACT(exp), PE(R) overlap per c-tile.
  - The S1 normalizer (colsum over q of E1T) is a ones[128,128]-weights
    matmul: every PSUM partition row carries the same colsum, so the
    reciprocal is directly partition-broadcast in SBUF - no DRAM bounce.
  - PSUM pools are phase-private (8 banks: 2 E2-S/transpose/cq, 2 E1T-S/csp,
    2 pa/pb, 1 rp, 1 s2sum) so batch b+1's front (Tile priority boost) never
    rotates behind batch b's tail allocations.
  - PE transposes run as 4-per-bank single accumulation groups (f32r identity,
    1.5 cyc/row) evacuated by one 512-row copy each.
"""

import numpy as np
from contextlib import ExitStack, nullcontext

import concourse.bass as bass
import concourse.mybir as mybir
import concourse.tile as tile
from concourse import bacc
from concourse.bass_utils import run_bass_kernel_spmd
from concourse.masks import make_identity

F32 = mybir.dt.float32
F32R = mybir.dt.float32r
I32 = mybir.dt.int32
AF = mybir.ActivationFunctionType
ALU = mybir.AluOpType
BF16 = mybir.dt.bfloat16

B, D, LC, LQ = 16, 128, 2048, 512
NCORES = 8
BL = B // NCORES          # batches per core
NEG = -1e30
NCT = LC // 128           # 16 c-tiles
NQT = LQ // 128           # 4 q-tiles
NCJ = LC // 512           # 4 c-chunks (free-dim)
HIPRI_OFF = 205
P8_BOOST = 40


def _build_nc():
    nc = bacc.Bacc("TRN2", target_bir_lowering=False)
    Ci = nc.dram_tensor("C", [BL, D, LC], F32, kind="ExternalInput")
    Qi = nc.dram_tensor("Q", [BL, D, LQ], F32, kind="ExternalInput")
    CMi = nc.dram_tensor("Cmask", [BL, LC], I32, kind="ExternalInput")
    QMi = nc.dram_tensor("Qmask", [BL, LQ], I32, kind="ExternalInput")
    w4C = nc.dram_tensor("w4C", [D, 1], F32, kind="ExternalInput")
    w4Q = nc.dram_tensor("w4Q", [D, 1], F32, kind="ExternalInput")
    w4mlu = nc.dram_tensor("w4mlu", [1, 1, D], F32, kind="ExternalInput")
    biasi = nc.dram_tensor("bias", [1], F32, kind="ExternalInput")
    out = nc.dram_tensor("out", [BL, 4 * D, LC], F32, kind="ExternalOutput")

    with tile.TileContext(nc) as tc, ExitStack() as ctx:
        const = ctx.enter_context(tc.tile_pool(name="const", bufs=1))
        sb2 = ctx.enter_context(tc.tile_pool(name="sb2", bufs=2))
        sb4 = ctx.enter_context(tc.tile_pool(name="sb4", bufs=4))
        # PSUM: 8 banks, phase-private pools (see module docstring)
        ps_7 = ctx.enter_context(tc.tile_pool(name="ps_7", bufs=2, space="PSUM"))
        ps_8 = ctx.enter_context(tc.tile_pool(name="ps_8", bufs=2, space="PSUM"))
        ps_ab = ctx.enter_context(tc.tile_pool(name="ps_ab", bufs=2, space="PSUM"))
        ps_r = ctx.enter_context(tc.tile_pool(name="ps_r", bufs=1, space="PSUM"))
        ps_cs = ctx.enter_context(tc.tile_pool(name="ps_cs", bufs=1, space="PSUM"))

        # ---- constants; identity first (Pool), then SWDGE w-consts so the
        #      weight loads don't sit behind identity building on Pool ----
        ident0 = const.tile([D, D], F32, name="ident0")
        make_identity(nc, ident0)
        identR = const.tile([D, D], F32R, name="identR")
        nc.vector.tensor_copy(identR, ident0)
        ones_f = const.tile([D, D], F32, name="ones_f")
        nc.vector.memset(ones_f, 1.0)
        onesR128 = const.tile([D, D], F32R, name="onesR128")
        nc.vector.tensor_copy(onesR128, ones_f)
        ones_colB = const.tile([D, 1], BF16, name="ones_colB")
        nc.vector.tensor_copy(ones_colB, ones_f[:, 0:1])
        wmlu_sb = const.tile([D, 1], F32, name="wmlu_sb")
        nc.gpsimd.dma_start(out=wmlu_sb, in_=w4mlu.ap().rearrange("a b d -> d (a b)"))
        w4C_sb = const.tile([D, 1], F32, name="w4C_sb")
        nc.gpsimd.dma_start(out=w4C_sb, in_=w4C[:, :])
        w4Q_sb = const.tile([D, 1], F32, name="w4Q_sb")
        nc.gpsimd.dma_start(out=w4Q_sb, in_=w4Q[:, :])
        bias_bc = const.tile([D, 1], F32, name="bias_bc")
        nc.scalar.dma_start(out=bias_bc, in_=biasi.ap().partition_broadcast(D))

        for b in range(BL):
            with (tc.high_priority(HIPRI_OFF) if b > 0 else nullcontext()):
                # ---- loads: Q, C-half0, masks, C-half1 ----
                Q_sb = sb2.tile([D, LQ], F32R, name="Q_sb")
                nc.sync.dma_start(out=Q_sb, in_=Qi[b, :, :].bitcast(F32R))
                C_sb = sb2.tile([D, LC], F32, name="C_sb")
                nc.sync.dma_start(out=C_sb[:, 0:1024], in_=Ci[b, :, 0:1024])
                cm_i = sb2.tile([128, NCT], I32, name="cm_i")
                nc.sync.dma_start(out=cm_i, in_=CMi[b, :].rearrange("(i p) -> p i", p=128))
                qm_i = sb2.tile([128, NQT], I32, name="qm_i")
                nc.sync.dma_start(out=qm_i, in_=QMi[b, :].rearrange("(i p) -> p i", p=128))
                nc.sync.dma_start(out=C_sb[:, 1024:2048], in_=Ci[b, :, 1024:2048])

                # ---- small prep ----
                Qw = sb2.tile([D, LQ], F32R, name="Qw")
                nc.vector.tensor_scalar_mul(Qw, Q_sb, wmlu_sb[:, 0:1])
                # mask -> NEG*(1-m):  m*(-NEG) + NEG
                cneg = sb2.tile([128, NCT], F32, name="cneg")
                nc.vector.tensor_scalar(cneg, cm_i, -NEG, NEG, op0=ALU.mult, op1=ALU.add)
                qneg = sb2.tile([128, NQT], F32, name="qneg")
                nc.vector.tensor_scalar(qneg, qm_i, -NEG, NEG, op0=ALU.mult, op1=ALU.add)
                # f32r matmul operands must be pre-rounded to f32r in SBUF
                CR = sb2.tile([D, LC], F32R, name="CR")
                for cj in range(NCJ):
                    eng = nc.gpsimd if cj % 2 == 0 else nc.vector
                    eng.tensor_copy(CR[:, cj * 512 : (cj + 1) * 512],
                                    C_sb[:, cj * 512 : (cj + 1) * 512])

                # ---- Q-only transposes first (C loads still in flight) ----
                Qt_sb = sb2.tile([128, NQT, 128], F32R, name="Qt_sb")
                tpq = ps_cs.tile([128, NQT, 128], F32R, name="cs")
                for qi in range(NQT):
                    nc.tensor.matmul(tpq[:, qi, :], Q_sb[:, qi * 128 : (qi + 1) * 128],
                                     identR, is_transpose=True,
                                     start=(qi == 0), stop=(qi == NQT - 1))
                nc.vector.tensor_copy(Qt_sb, tpq)

                # c0[c] = sum_d C[d,c] w4C[d]; q1[q] = sum_d Q[d,q] w4Q[d]
                cq_p = ps_7.tile([128, NCT + NQT], F32, name="s7")
                for qi in range(NQT):
                    nc.tensor.matmul(cq_p[:, NCT + qi : NCT + qi + 1],
                                     Q_sb.bitcast(F32)[:, qi * 128 : (qi + 1) * 128],
                                     w4Q_sb, start=True, stop=True)
                for ci in range(NCT // 2):
                    nc.tensor.matmul(cq_p[:, ci : ci + 1],
                                     C_sb[:, ci * 128 : (ci + 1) * 128],
                                     w4C_sb, start=True, stop=True)
                bias_c = sb2.tile([128, NCT], F32, name="bias_c")
                nc.vector.tensor_tensor(bias_c[:, 0 : NCT // 2], cq_p[:, 0 : NCT // 2],
                                        cneg[:, 0 : NCT // 2], ALU.add)
                bias_q0 = sb2.tile([128, NQT], F32, name="bias_q0")
                nc.vector.tensor_tensor(bias_q0, cq_p[:, NCT : NCT + NQT], qneg, ALU.add)
                bias_q = sb2.tile([128, NQT], F32, name="bias_q")
                nc.vector.tensor_scalar_add(bias_q, bias_q0, bias_bc[:, 0:1])

                Ct_sb = sb2.tile([128, NCT, 128], BF16, name="Ct_sb")

                def transpose_quad(g):
                    tp = ps_cs.tile([128, 4, 128], F32R, name="cs")
                    for k in range(4):
                        ci = 4 * g + k
                        nc.tensor.matmul(tp[:, k, :], CR[:, ci * 128 : (ci + 1) * 128],
                                         identR, is_transpose=True,
                                         start=(k == 0), stop=(k == 3))
                    nc.vector.tensor_copy(Ct_sb[:, 4 * g : 4 * g + 4, :], tp)

                transpose_quad(0)
                transpose_quad(1)
                transpose_quad(2)
                transpose_quad(3)

                # ---- E2 = exp(S + c-terms) in [c-part, q-free], fused with
                #      R[q,d] += E2[c,q]^T Ct[c,d] and s2sum[q] += colsum ----
                E2 = sb2.tile([128, NCT, LQ], BF16, name="E2")
                rp = ps_r.tile([128, NQT, 128], F32, name="rp")
                s2pp = ps_cs.tile([128, NQT], F32, name="cs")
                for ci in range(NCT):
                    sp = ps_7.tile([128, LQ], F32, name="s7")
                    nc.tensor.matmul(sp, CR[:, ci * 128 : (ci + 1) * 128], Qw,
                                     start=True, stop=True)
                    nc.scalar.activation(E2[:, ci, :], sp, AF.Exp,
                                         bias=bias_c[:, ci : ci + 1], scale=1.0)
                    for qi in range(NQT):
                        lhs = E2[:, ci, qi * 128 : (qi + 1) * 128]
                        first = ci == 0 and qi == 0
                        last = ci == NCT - 1 and qi == NQT - 1
                        nc.tensor.matmul(rp[:, qi, :], lhs, Ct_sb[:, ci, :],
                                         start=first, stop=last)
                        nc.tensor.matmul(s2pp[:, qi : qi + 1], lhs, ones_colB,
                                         start=first, stop=last)
                    if ci == 1:
                        # C-half-1-dependent front matter, emitted late so its
                        # waits don't head-of-line block the PE queue
                        for cil in range(NCT // 2, NCT):
                            nc.tensor.matmul(cq_p[:, cil : cil + 1],
                                             C_sb[:, cil * 128 : (cil + 1) * 128],
                                             w4C_sb, start=True, stop=True)
                        nc.vector.tensor_tensor(bias_c[:, NCT // 2 : NCT],
                                                cq_p[:, NCT // 2 : NCT],
                                                cneg[:, NCT // 2 : NCT], ALU.add)


            # ---- phase 8: E1T = exp(S^T + q-terms) in [q-part, c-free];
            #      per 512-col c-chunk: colsum -> broadcast rcp, A^T, B^T ----
            # R normalize sits here in program order so it doesn't head-of-line
            # block the DVE queue while the rp accumulation is still open.
            rs2 = sb2.tile([128, NQT], F32, name="rs2")
            nc.vector.reciprocal(rs2, s2pp)
            R_sb = sb2.tile([128, NQT, 128], F32R, name="R_sb")
            for qi in range(NQT):
                nc.vector.tensor_scalar_mul(R_sb[:, qi, :], rp[:, qi, :],
                                            rs2[:, qi : qi + 1])
            # C pass-through store: mid-kernel DMA lull, away from loads/tail
            nc.scalar.dma_start(out=out[b, 0:128, :], in_=C_sb)

            E1T = sb2.tile([128, NQT, LC], F32R, name="E1T")

            def consume(cj):
                sl = slice(cj * 512, (cj + 1) * 512)
                csp = ps_ab.tile([128, 512], F32, name="pab")
                for qi in range(NQT):
                    nc.tensor.matmul(csp, onesR128, E1T[:, qi, sl],
                                     start=(qi == 0), stop=(qi == NQT - 1))
                rcp_c = sb4.tile([128, 512], F32, name="rcp_c")
                nc.vector.reciprocal(rcp_c, csp)
                pa = ps_ab.tile([128, 512], F32, name="pab")
                for qi in range(NQT):
                    nc.tensor.matmul(pa, Qt_sb[:, qi, :], E1T[:, qi, sl],
                                     start=(qi == 0), stop=(qi == NQT - 1))
                pb = ps_ab.tile([128, 512], F32, name="pab")
                for qi in range(NQT):
                    nc.tensor.matmul(pb, R_sb[:, qi, :], E1T[:, qi, sl],
                                     start=(qi == 0), stop=(qi == NQT - 1))
                ACB = sb2.tile([128, 3, 512], F32, name="ACB")
                Bt_t = sb2.tile([128, 512], F32, name="Bt_t")
                nc.vector.tensor_tensor(ACB[:, 0, :], pa, rcp_c, ALU.mult)
                nc.vector.tensor_tensor(Bt_t, pb, rcp_c, ALU.mult)
                e1, e2 = (nc.vector, nc.gpsimd) if cj % 2 == 0 else (nc.gpsimd, nc.vector)
                e1.tensor_tensor(ACB[:, 1, :], C_sb[:, sl], ACB[:, 0, :], ALU.mult)
                e2.tensor_tensor(ACB[:, 2, :], C_sb[:, sl], Bt_t, ALU.mult)
                # one DMA stores [At|CA|CB] for this chunk: rows 128:512.
                # Last batch: split so the A rows ship before CA/CB finish.
                if b == BL - 1:
                    nc.sync.dma_start(out=out[b, 128:256, sl], in_=ACB[:, 0, :])
                    nc.sync.dma_start(out=out[b, 256:384, sl], in_=ACB[:, 1, :])
                    nc.sync.dma_start(out=out[b, 384:512, sl], in_=ACB[:, 2, :])
                else:
                    q = nc.sync if cj % 2 == 0 else nc.scalar
                    q.dma_start(
                        out=out[b, 128:512, sl].rearrange("(r p) c -> p r c", p=128),
                        in_=ACB,
                    )

            for cj in range(NCJ):
                sl = slice(cj * 512, (cj + 1) * 512)
                # first chunk's S/exp outranks the phase-7 tail so ACT rolls
                # straight from E2 into E1T exps at the phase boundary
                with (tc.high_priority(P8_BOOST) if cj == 0 else nullcontext()):
                    for qi in range(NQT):
                        sp = ps_8.tile([128, 512], F32, name="s8")
                        nc.tensor.matmul(sp, Qw[:, qi * 128 : (qi + 1) * 128],
                                         CR[:, sl], start=True, stop=True)
                        nc.scalar.activation(E1T[:, qi, sl], sp, AF.Exp,
                                             bias=bias_q[:, qi : qi + 1], scale=1.0)
                if cj > 0:
                    consume(cj - 1)
            consume(NCJ - 1)

    nc.finalize()
    return nc


_NC = None


def _get_nc():
    global _NC
    if _NC is None:
        _NC = _build_nc()
    return _NC


def kernel(C, Q, Cmask, Qmask, w4C, w4Q, w4mlu, bias, _trace=False):
    C = np.ascontiguousarray(np.asarray(C, dtype=np.float32))
    Q = np.ascontiguousarray(np.asarray(Q, dtype=np.float32))
    Cmask = np.ascontiguousarray(np.asarray(Cmask, dtype=np.int32))
    Qmask = np.ascontiguousarray(np.asarray(Qmask, dtype=np.int32))
    w4C = np.ascontiguousarray(np.asarray(w4C, dtype=np.float32))
    w4Q = np.ascontiguousarray(np.asarray(w4Q, dtype=np.float32))
    w4mlu = np.ascontiguousarray(np.asarray(w4mlu, dtype=np.float32))
    bias = np.ascontiguousarray(np.asarray(bias, dtype=np.float32))

    nc = _get_nc()
    in_maps = []
    for i in range(NCORES):
        s = slice(i * BL, (i + 1) * BL)
        in_maps.append({
            "C": C[s], "Q": Q[s], "Cmask": Cmask[s], "Qmask": Qmask[s],
            "w4C": w4C, "w4Q": w4Q, "w4mlu": w4mlu, "bias": bias,
        })
    res = run_bass_kernel_spmd(nc, in_maps, core_ids=list(range(NCORES)),
                               trace=_trace)
    out = np.concatenate([r["out"] for r in res.results], axis=0)
    if _trace:
        kernel._last_results = res
    return out


# revision 37
# speedup vs baseline: 1.1473x; 1.0130x over previous
"""CQAttention Trainium2 kernel.

Reference computation per batch b (C:[D,Lc], Q:[D,Lq], D=128, Lc=2048, Lq=512):
    Ct = C^T, Qt = Q^T
    S  = Ct@w4C + (Qt@w4Q)^T + (Ct*w4mlu)@Qt^T + bias        [Lc, Lq]
    S1 = softmax_q(S + NEG*(1-qmask))                         (over Lq)
    S2 = softmax_c(S + NEG*(1-cmask))                         (over Lc)
    A  = S1 @ Qt                                              [Lc, D]
    B  = S1 @ (S2^T @ Ct)     (right-assoc of (S1 S2^T) Ct)   [Lc, D]
    out= transpose(concat([Ct, A, Ct*A, Ct*B], -1))           [4D, Lc]

Kernel strategy (pure data parallel over batch: 16 batches / 8 cores):
  - S is never materialized: two matmul families compute S(sub2-part) in
    [c-part,q-free] and [q-part,c-free] layouts straight into PSUM; ScalarE
    exp() reads PSUM with a per-partition bias AP folding the softmax-relevant
    affine terms (terms constant along the softmax axis cancel). sub2 uses
    Qw = Q*w4mlu, so C needs no scaled copy (C^T diag(w) Q == C^T (w * Q)).
  - Tile-pipelined phases: each E2=[c,q] exp tile immediately feeds
    R += E2^T@Ct (whole-bank single PSUM accumulation group across all 4
    q-tiles x 16 c-tiles) plus a paired N=1 ones-matmul accumulating the S2
    normalizer in its own bank; PE(S), ACT(exp), PE(R) overlap per c-tile.
  - The S1 normalizer (colsum over q of E1T) is a ones[128,128]-weights
    matmul: every PSUM partition row carries the same colsum, so the
    reciprocal is directly partition-broadcast in SBUF - no DRAM bounce.
  - PSUM pools are phase-private (8 banks: 2 E2-S/transpose/cq, 2 E1T-S/csp,
    2 pa/pb, 1 rp, 1 s2sum) so batch b+1's front (Tile priority boost) never
    rotates behind batch b's tail allocations.
  - PE transposes run as 4-per-bank single accumulation groups (f32r identity,
    1.5 cyc/row) evacuated by one 512-row copy each.
"""

import numpy as np
from contextlib import ExitStack, nullcontext

import concourse.bass as bass
import concourse.mybir as mybir
import concourse.tile as tile
from concourse import bacc
from concourse.bass_utils import run_bass_kernel_spmd
from concourse.masks import make_identity

F32 = mybir.dt.float32
F32R = mybir.dt.float32r
I32 = mybir.dt.int32
AF = mybir.ActivationFunctionType
ALU = mybir.AluOpType
BF16 = mybir.dt.bfloat16

B, D, LC, LQ = 16, 128, 2048, 512
NCORES = 8
BL = B // NCORES          # batches per core
NEG = -1e30
NCT = LC // 128           # 16 c-tiles
NQT = LQ // 128           # 4 q-tiles
NCJ = LC // 512           # 4 c-chunks (free-dim)
HIPRI_OFF = 245
P8_BOOST = 40


def _build_nc():
    nc = bacc.Bacc("TRN2", target_bir_lowering=False)
    Ci = nc.dram_tensor("C", [BL, D, LC], F32, kind="ExternalInput")
    Qi = nc.dram_tensor("Q", [BL, D, LQ], F32, kind="ExternalInput")
    CMi = nc.dram_tensor("Cmask", [BL, LC], I32, kind="ExternalInput")
    QMi = nc.dram_tensor("Qmask", [BL, LQ], I32, kind="ExternalInput")
    w4C = nc.dram_tensor("w4C", [D, 1], F32, kind="ExternalInput")
    w4Q = nc.dram_tensor("w4Q", [D, 1], F32, kind="ExternalInput")
    w4mlu = nc.dram_tensor("w4mlu", [1, 1, D], F32, kind="ExternalInput")
    biasi = nc.dram_tensor("bias", [1], F32, kind="ExternalInput")
    out = nc.dram_tensor("out", [BL, 4 * D, LC], F32, kind="ExternalOutput")

    with tile.TileContext(nc) as tc, ExitStack() as ctx:
        const = ctx.enter_context(tc.tile_pool(name="const", bufs=1))
        sb2 = ctx.enter_context(tc.tile_pool(name="sb2", bufs=2))
        sb4 = ctx.enter_context(tc.tile_pool(name="sb4", bufs=4))
        # PSUM: 8 banks, phase-private pools (see module docstring)
        ps_7 = ctx.enter_context(tc.tile_pool(name="ps_7", bufs=2, space="PSUM"))
        ps_8 = ctx.enter_context(tc.tile_pool(name="ps_8", bufs=2, space="PSUM"))
        ps_ab = ctx.enter_context(tc.tile_pool(name="ps_ab", bufs=2, space="PSUM"))
        ps_r = ctx.enter_context(tc.tile_pool(name="ps_r", bufs=1, space="PSUM"))
        ps_cs = ctx.enter_context(tc.tile_pool(name="ps_cs", bufs=1, space="PSUM"))

        # ---- constants; identity first (Pool), then SWDGE w-consts so the
        #      weight loads don't sit behind identity building on Pool ----
        ident0 = const.tile([D, D], F32, name="ident0")
        make_identity(nc, ident0)
        identR = const.tile([D, D], F32R, name="identR")
        nc.vector.tensor_copy(identR, ident0)
        ones_f = const.tile([D, D], F32, name="ones_f")
        nc.vector.memset(ones_f, 1.0)
        onesR128 = const.tile([D, D], F32R, name="onesR128")
        nc.vector.tensor_copy(onesR128, ones_f)
        ones_colB = const.tile([D, 1], BF16, name="ones_colB")
        nc.vector.tensor_copy(ones_colB, ones_f[:, 0:1])
        wmlu_sb = const.tile([D, 1], F32, name="wmlu_sb")
        nc.gpsimd.dma_start(out=wmlu_sb, in_=w4mlu.ap().rearrange("a b d -> d (a b)"))
        w4C_sb = const.tile([D, 1], F32, name="w4C_sb")
        nc.gpsimd.dma_start(out=w4C_sb, in_=w4C[:, :])
        w4Q_sb = const.tile([D, 1], F32, name="w4Q_sb")
        nc.gpsimd.dma_start(out=w4Q_sb, in_=w4Q[:, :])
        bias_bc = const.tile([D, 1], F32, name="bias_bc")
        nc.scalar.dma_start(out=bias_bc, in_=biasi.ap().partition_broadcast(D))

        for b in range(BL):
            with (tc.high_priority(HIPRI_OFF) if b > 0 else nullcontext()):
                # ---- loads: Q, C-half0, masks, C-half1 ----
                Q_sb = sb2.tile([D, LQ], F32R, name="Q_sb")
                nc.sync.dma_start(out=Q_sb, in_=Qi[b, :, :].bitcast(F32R))
                C_sb = sb2.tile([D, LC], F32, name="C_sb")
                nc.sync.dma_start(out=C_sb[:, 0:1024], in_=Ci[b, :, 0:1024])
                cm_i = sb2.tile([128, NCT], I32, name="cm_i")
                nc.sync.dma_start(out=cm_i, in_=CMi[b, :].rearrange("(i p) -> p i", p=128))
                qm_i = sb2.tile([128, NQT], I32, name="qm_i")
                nc.sync.dma_start(out=qm_i, in_=QMi[b, :].rearrange("(i p) -> p i", p=128))
                nc.sync.dma_start(out=C_sb[:, 1024:2048], in_=Ci[b, :, 1024:2048])

                # ---- small prep ----
                Qw = sb2.tile([D, LQ], F32R, name="Qw")
                nc.vector.tensor_scalar_mul(Qw, Q_sb, wmlu_sb[:, 0:1])
                # mask -> NEG*(1-m):  m*(-NEG) + NEG
                cneg = sb2.tile([128, NCT], F32, name="cneg")
                nc.vector.tensor_scalar(cneg, cm_i, -NEG, NEG, op0=ALU.mult, op1=ALU.add)
                qneg = sb2.tile([128, NQT], F32, name="qneg")
                nc.vector.tensor_scalar(qneg, qm_i, -NEG, NEG, op0=ALU.mult, op1=ALU.add)
                # f32r matmul operands must be pre-rounded to f32r in SBUF
                CR = sb2.tile([D, LC], F32R, name="CR")
                for cj in range(NCJ):
                    eng = nc.gpsimd if cj % 2 == 0 else nc.vector
                    eng.tensor_copy(CR[:, cj * 512 : (cj + 1) * 512],
                                    C_sb[:, cj * 512 : (cj + 1) * 512])

                # ---- Q-only transposes first (C loads still in flight) ----
                Qt_sb = sb2.tile([128, NQT, 128], F32R, name="Qt_sb")
                tpq = ps_cs.tile([128, NQT, 128], F32R, name="cs")
                for qi in range(NQT):
                    nc.tensor.matmul(tpq[:, qi, :], Q_sb[:, qi * 128 : (qi + 1) * 128],
                                     identR, is_transpose=True,
                                     start=(qi == 0), stop=(qi == NQT - 1))
                nc.vector.tensor_copy(Qt_sb, tpq)

                # c0[c] = sum_d C[d,c] w4C[d]; q1[q] = sum_d Q[d,q] w4Q[d]
                cq_p = ps_7.tile([128, NCT + NQT], F32, name="s7")
                for qi in range(NQT):
                    nc.tensor.matmul(cq_p[:, NCT + qi : NCT + qi + 1],
                                     Q_sb.bitcast(F32)[:, qi * 128 : (qi + 1) * 128],
                                     w4Q_sb, start=True, stop=True)
                for ci in range(NCT // 2):
                    nc.tensor.matmul(cq_p[:, ci : ci + 1],
                                     C_sb[:, ci * 128 : (ci + 1) * 128],
                                     w4C_sb, start=True, stop=True)
                bias_c = sb2.tile([128, NCT], F32, name="bias_c")
                nc.vector.tensor_tensor(bias_c[:, 0 : NCT // 2], cq_p[:, 0 : NCT // 2],
                                        cneg[:, 0 : NCT // 2], ALU.add)
                bias_q0 = sb2.tile([128, NQT], F32, name="bias_q0")
                nc.vector.tensor_tensor(bias_q0, cq_p[:, NCT : NCT + NQT], qneg, ALU.add)
                bias_q = sb2.tile([128, NQT], F32, name="bias_q")
                nc.vector.tensor_scalar_add(bias_q, bias_q0, bias_bc[:, 0:1])

                Ct_sb = sb2.tile([128, NCT, 128], BF16, name="Ct_sb")

                def transpose_quad(g):
                    tp = ps_cs.tile([128, 4, 128], F32R, name="cs")
                    for k in range(4):
                        ci = 4 * g + k
                        nc.tensor.matmul(tp[:, k, :], CR[:, ci * 128 : (ci + 1) * 128],
                                         identR, is_transpose=True,
                                         start=(k == 0), stop=(k == 3))
                    nc.vector.tensor_copy(Ct_sb[:, 4 * g : 4 * g + 4, :], tp)

                transpose_quad(0)
                transpose_quad(1)
                transpose_quad(2)
                transpose_quad(3)

                # ---- E2 = exp(S + c-terms) in [c-part, q-free], fused with
                #      R[q,d] += E2[c,q]^T Ct[c,d] and s2sum[q] += colsum ----
                E2 = sb2.tile([128, NCT, LQ], BF16, name="E2")
                rp = ps_r.tile([128, NQT, 128], F32, name="rp")
                s2pp = ps_cs.tile([128, NQT], F32, name="cs")
                for ci in range(NCT):
                    sp = ps_7.tile([128, LQ], F32, name="s7")
                    nc.tensor.matmul(sp, CR[:, ci * 128 : (ci + 1) * 128], Qw,
                                     start=True, stop=True)
                    nc.scalar.activation(E2[:, ci, :], sp, AF.Exp,
                                         bias=bias_c[:, ci : ci + 1], scale=1.0)
                    for qi in range(NQT):
                        lhs = E2[:, ci, qi * 128 : (qi + 1) * 128]
                        first = ci == 0 and qi == 0
                        last = ci == NCT - 1 and qi == NQT - 1
                        nc.tensor.matmul(rp[:, qi, :], lhs, Ct_sb[:, ci, :],
                                         start=first, stop=last)
                        nc.tensor.matmul(s2pp[:, qi : qi + 1], lhs, ones_colB,
                                         start=first, stop=last)
                    if ci == 1:
                        # C-half-1-dependent front matter, emitted late so its
                        # waits don't head-of-line block the PE queue
                        for cil in range(NCT // 2, NCT):
                            nc.tensor.matmul(cq_p[:, cil : cil + 1],
                                             C_sb[:, cil * 128 : (cil + 1) * 128],
                                             w4C_sb, start=True, stop=True)
                        nc.vector.tensor_tensor(bias_c[:, NCT // 2 : NCT],
                                                cq_p[:, NCT // 2 : NCT],
                                                cneg[:, NCT // 2 : NCT], ALU.add)


            # ---- phase 8: E1T = exp(S^T + q-terms) in [q-part, c-free];
            #      per 512-col c-chunk: colsum -> broadcast rcp, A^T, B^T ----
            # R normalize sits here in program order so it doesn't head-of-line
            # block the DVE queue while the rp accumulation is still open.
            rs2 = sb2.tile([128, NQT], F32, name="rs2")
            nc.vector.reciprocal(rs2, s2pp)
            R_sb = sb2.tile([128, NQT, 128], F32R, name="R_sb")
            for qi in range(NQT):
                nc.vector.tensor_scalar_mul(R_sb[:, qi, :], rp[:, qi, :],
                                            rs2[:, qi : qi + 1])
            # C pass-through store: mid-kernel DMA lull, away from loads/tail
            nc.scalar.dma_start(out=out[b, 0:128, :], in_=C_sb)

            E1T = sb2.tile([128, NQT, LC], F32R, name="E1T")

            def consume(cj):
                sl = slice(cj * 512, (cj + 1) * 512)
                csp = ps_ab.tile([128, 512], F32, name="pab")
                for qi in range(NQT):
                    nc.tensor.matmul(csp, onesR128, E1T[:, qi, sl],
                                     start=(qi == 0), stop=(qi == NQT - 1))
                rcp_c = sb4.tile([128, 512], F32, name="rcp_c")
                nc.vector.reciprocal(rcp_c, csp)
                pa = ps_ab.tile([128, 512], F32, name="pab")
                for qi in range(NQT):
                    nc.tensor.matmul(pa, Qt_sb[:, qi, :], E1T[:, qi, sl],
                                     start=(qi == 0), stop=(qi == NQT - 1))
                pb = ps_ab.tile([128, 512], F32, name="pab")
                for qi in range(NQT):
                    nc.tensor.matmul(pb, R_sb[:, qi, :], E1T[:, qi, sl],
                                     start=(qi == 0), stop=(qi == NQT - 1))
                ACB = sb2.tile([128, 3, 512], F32, name="ACB")
                Bt_t = sb2.tile([128, 512], F32, name="Bt_t")
                nc.vector.tensor_tensor(ACB[:, 0, :], pa, rcp_c, ALU.mult)
                nc.vector.tensor_tensor(Bt_t, pb, rcp_c, ALU.mult)
                e1, e2 = (nc.vector, nc.gpsimd) if cj % 2 == 0 else (nc.gpsimd, nc.vector)
                e1.tensor_tensor(ACB[:, 1, :], C_sb[:, sl], ACB[:, 0, :], ALU.mult)
                e2.tensor_tensor(ACB[:, 2, :], C_sb[:, sl], Bt_t, ALU.mult)
                # one DMA stores [At|CA|CB] for this chunk: rows 128:512.
                # Last batch: split so the A rows ship before CA/CB finish.
                if b == BL - 1:
                    nc.sync.dma_start(out=out[b, 128:256, sl], in_=ACB[:, 0, :])
                    nc.sync.dma_start(out=out[b, 256:384, sl], in_=ACB[:, 1, :])
                    nc.sync.dma_start(out=out[b, 384:512, sl], in_=ACB[:, 2, :])
                else:
                    q = nc.sync if cj % 2 == 0 else nc.scalar
                    q.dma_start(
                        out=out[b, 128:512, sl].rearrange("(r p) c -> p r c", p=128),
                        in_=ACB,
                    )

            for cj in range(NCJ):
                sl = slice(cj * 512, (cj + 1) * 512)
                # first chunk's S/exp outranks the phase-7 tail so ACT rolls
                # straight from E2 into E1T exps at the phase boundary
                with (tc.high_priority(P8_BOOST) if cj == 0 else nullcontext()):
                    for qi in range(NQT):
                        sp = ps_8.tile([128, 512], F32, name="s8")
                        nc.tensor.matmul(sp, Qw[:, qi * 128 : (qi + 1) * 128],
                                         CR[:, sl], start=True, stop=True)
                        nc.scalar.activation(E1T[:, qi, sl], sp, AF.Exp,
                                             bias=bias_q[:, qi : qi + 1], scale=1.0)
                if cj > 0:
                    consume(cj - 1)
            consume(NCJ - 1)

    nc.finalize()
    return nc


_NC = None


def _get_nc():
    global _NC
    if _NC is None:
        _NC = _build_nc()
    return _NC


def kernel(C, Q, Cmask, Qmask, w4C, w4Q, w4mlu, bias, _trace=False):
    C = np.ascontiguousarray(np.asarray(C, dtype=np.float32))
    Q = np.ascontiguousarray(np.asarray(Q, dtype=np.float32))
    Cmask = np.ascontiguousarray(np.asarray(Cmask, dtype=np.int32))
    Qmask = np.ascontiguousarray(np.asarray(Qmask, dtype=np.int32))
    w4C = np.ascontiguousarray(np.asarray(w4C, dtype=np.float32))
    w4Q = np.ascontiguousarray(np.asarray(w4Q, dtype=np.float32))
    w4mlu = np.ascontiguousarray(np.asarray(w4mlu, dtype=np.float32))
    bias = np.ascontiguousarray(np.asarray(bias, dtype=np.float32))

    nc = _get_nc()
    in_maps = []
    for i in range(NCORES):
        s = slice(i * BL, (i + 1) * BL)
        in_maps.append({
            "C": C[s], "Q": Q[s], "Cmask": Cmask[s], "Qmask": Qmask[s],
            "w4C": w4C, "w4Q": w4Q, "w4mlu": w4mlu, "bias": bias,
        })
    res = run_bass_kernel_spmd(nc, in_maps, core_ids=list(range(NCORES)),
                               trace=_trace)
    out = np.concatenate([r["out"] for r in res.results], axis=0)
    if _trace:
        kernel._last_results = res
    return out


# revision 38
# speedup vs baseline: 1.1617x; 1.0125x over previous
"""CQAttention Trainium2 kernel.

Reference computation per batch b (C:[D,Lc], Q:[D,Lq], D=128, Lc=2048, Lq=512):
    Ct = C^T, Qt = Q^T
    S  = Ct@w4C + (Qt@w4Q)^T + (Ct*w4mlu)@Qt^T + bias        [Lc, Lq]
    S1 = softmax_q(S + NEG*(1-qmask))                         (over Lq)
    S2 = softmax_c(S + NEG*(1-cmask))                         (over Lc)
    A  = S1 @ Qt                                              [Lc, D]
    B  = S1 @ (S2^T @ Ct)     (right-assoc of (S1 S2^T) Ct)   [Lc, D]
    out= transpose(concat([Ct, A, Ct*A, Ct*B], -1))           [4D, Lc]

Kernel strategy (pure data parallel over batch: 16 batches / 8 cores):
  - S is never materialized: two matmul families compute S(sub2-part) in
    [c-part,q-free] and [q-part,c-free] layouts straight into PSUM; ScalarE
    exp() reads PSUM with a per-partition bias AP folding the softmax-relevant
    affine terms (terms constant along the softmax axis cancel). sub2 uses
    Qw = Q*w4mlu, so C needs no scaled copy (C^T diag(w) Q == C^T (w * Q)).
  - Tile-pipelined phases: each E2=[c,q] exp tile immediately feeds
    R += E2^T@Ct (whole-bank single PSUM accumulation group across all 4
    q-tiles x 16 c-tiles) plus a paired N=1 ones-matmul accumulating the S2
    normalizer in its own bank; PE(S), ACT(exp), PE(R) overlap per c-tile.
  - The S1 normalizer (colsum over q of E1T) is a ones[128,128]-weights
    matmul: every PSUM partition row carries the same colsum, so the
    reciprocal is directly partition-broadcast in SBUF - no DRAM bounce.
  - PSUM pools are phase-private (8 banks: 2 E2-S/transpose/cq, 2 E1T-S/csp,
    2 pa/pb, 1 rp, 1 s2sum) so batch b+1's front (Tile priority boost) never
    rotates behind batch b's tail allocations.
  - PE transposes run as 4-per-bank single accumulation groups (f32r identity,
    1.5 cyc/row) evacuated by one 512-row copy each.
"""

import numpy as np
from contextlib import ExitStack, nullcontext

import concourse.bass as bass
import concourse.mybir as mybir
import concourse.tile as tile
from concourse import bacc
from concourse.bass_utils import run_bass_kernel_spmd
from concourse.masks import make_identity

F32 = mybir.dt.float32
F32R = mybir.dt.float32r
I32 = mybir.dt.int32
AF = mybir.ActivationFunctionType
ALU = mybir.AluOpType
BF16 = mybir.dt.bfloat16

B, D, LC, LQ = 16, 128, 2048, 512
NCORES = 8
BL = B // NCORES          # batches per core
NEG = -1e30
NCT = LC // 128           # 16 c-tiles
NQT = LQ // 128           # 4 q-tiles
NCJ = LC // 512           # 4 c-chunks (free-dim)
HIPRI_OFF = 245
P8_BOOST = 40


def _build_nc():
    nc = bacc.Bacc("TRN2", target_bir_lowering=False)
    Ci = nc.dram_tensor("C", [BL, D, LC], F32, kind="ExternalInput")
    Qi = nc.dram_tensor("Q", [BL, D, LQ], F32, kind="ExternalInput")
    CMi = nc.dram_tensor("Cmask", [BL, LC], I32, kind="ExternalInput")
    QMi = nc.dram_tensor("Qmask", [BL, LQ], I32, kind="ExternalInput")
    w4C = nc.dram_tensor("w4C", [D, 1], F32, kind="ExternalInput")
    w4Q = nc.dram_tensor("w4Q", [D, 1], F32, kind="ExternalInput")
    w4mlu = nc.dram_tensor("w4mlu", [1, 1, D], F32, kind="ExternalInput")
    biasi = nc.dram_tensor("bias", [1], F32, kind="ExternalInput")
    out = nc.dram_tensor("out", [BL, 4 * D, LC], F32, kind="ExternalOutput")

    with tile.TileContext(nc) as tc, ExitStack() as ctx:
        const = ctx.enter_context(tc.tile_pool(name="const", bufs=1))
        sb2 = ctx.enter_context(tc.tile_pool(name="sb2", bufs=2))
        sb4 = ctx.enter_context(tc.tile_pool(name="sb4", bufs=4))
        # PSUM: 8 banks, phase-private pools (see module docstring)
        ps_7 = ctx.enter_context(tc.tile_pool(name="ps_7", bufs=2, space="PSUM"))
        ps_8 = ctx.enter_context(tc.tile_pool(name="ps_8", bufs=2, space="PSUM"))
        ps_ab = ctx.enter_context(tc.tile_pool(name="ps_ab", bufs=2, space="PSUM"))
        ps_r = ctx.enter_context(tc.tile_pool(name="ps_r", bufs=1, space="PSUM"))
        ps_cs = ctx.enter_context(tc.tile_pool(name="ps_cs", bufs=1, space="PSUM"))

        # ---- constants; identity first (Pool), then SWDGE w-consts so the
        #      weight loads don't sit behind identity building on Pool ----
        ident0 = const.tile([D, D], F32, name="ident0")
        make_identity(nc, ident0)
        identR = const.tile([D, D], F32R, name="identR")
        nc.vector.tensor_copy(identR, ident0)
        ones_f = const.tile([D, D], F32, name="ones_f")
        nc.vector.memset(ones_f, 1.0)
        onesR128 = const.tile([D, D], F32R, name="onesR128")
        nc.vector.tensor_copy(onesR128, ones_f)
        ones_colB = const.tile([D, 1], BF16, name="ones_colB")
        nc.vector.tensor_copy(ones_colB, ones_f[:, 0:1])
        wmlu_sb = const.tile([D, 1], F32, name="wmlu_sb")
        nc.gpsimd.dma_start(out=wmlu_sb, in_=w4mlu.ap().rearrange("a b d -> d (a b)"))
        w4C_sb = const.tile([D, 1], F32, name="w4C_sb")
        nc.gpsimd.dma_start(out=w4C_sb, in_=w4C[:, :])
        w4Q_sb = const.tile([D, 1], F32, name="w4Q_sb")
        nc.gpsimd.dma_start(out=w4Q_sb, in_=w4Q[:, :])
        bias_bc = const.tile([D, 1], F32, name="bias_bc")
        nc.scalar.dma_start(out=bias_bc, in_=biasi.ap().partition_broadcast(D))

        for b in range(BL):
            with (tc.high_priority(HIPRI_OFF) if b > 0 else nullcontext()):
                # ---- loads: Q, C-half0, masks, C-half1 ----
                Q_sb = sb2.tile([D, LQ], F32R, name="Q_sb")
                nc.sync.dma_start(out=Q_sb, in_=Qi[b, :, :].bitcast(F32R))
                C_sb = sb2.tile([D, LC], F32, name="C_sb")
                nc.sync.dma_start(out=C_sb[:, 0:512], in_=Ci[b, :, 0:512])
                cm_i = sb2.tile([128, NCT], I32, name="cm_i")
                nc.sync.dma_start(out=cm_i, in_=CMi[b, :].rearrange("(i p) -> p i", p=128))
                nc.sync.dma_start(out=C_sb[:, 512:1024], in_=Ci[b, :, 512:1024])
                qm_i = sb2.tile([128, NQT], I32, name="qm_i")
                nc.sync.dma_start(out=qm_i, in_=QMi[b, :].rearrange("(i p) -> p i", p=128))
                nc.sync.dma_start(out=C_sb[:, 1024:2048], in_=Ci[b, :, 1024:2048])

                # ---- small prep ----
                Qw = sb2.tile([D, LQ], F32R, name="Qw")
                nc.vector.tensor_scalar_mul(Qw, Q_sb, wmlu_sb[:, 0:1])
                # mask -> NEG*(1-m):  m*(-NEG) + NEG
                cneg = sb2.tile([128, NCT], F32, name="cneg")
                nc.vector.tensor_scalar(cneg, cm_i, -NEG, NEG, op0=ALU.mult, op1=ALU.add)
                qneg = sb2.tile([128, NQT], F32, name="qneg")
                nc.vector.tensor_scalar(qneg, qm_i, -NEG, NEG, op0=ALU.mult, op1=ALU.add)
                # f32r matmul operands must be pre-rounded to f32r in SBUF;
                # chunk 0 on the (idle-early) ACT engine so the first S-matmul
                # isn't gated by Pool's const DMAs
                CR = sb2.tile([D, LC], F32R, name="CR")
                for cj in range(NCJ):
                    sl0 = slice(cj * 512, (cj + 1) * 512)
                    if cj == 0:
                        nc.scalar.copy(CR[:, sl0], C_sb[:, sl0])
                    else:
                        eng = nc.gpsimd if cj % 2 == 0 else nc.vector
                        eng.tensor_copy(CR[:, sl0], C_sb[:, sl0])

                # ---- Q-only transposes first (C loads still in flight) ----
                Qt_sb = sb2.tile([128, NQT, 128], F32R, name="Qt_sb")
                tpq = ps_cs.tile([128, NQT, 128], F32R, name="cs")
                for qi in range(NQT):
                    nc.tensor.matmul(tpq[:, qi, :], Q_sb[:, qi * 128 : (qi + 1) * 128],
                                     identR, is_transpose=True,
                                     start=(qi == 0), stop=(qi == NQT - 1))
                nc.vector.tensor_copy(Qt_sb, tpq)

                # c0[c] = sum_d C[d,c] w4C[d]; q1[q] = sum_d Q[d,q] w4Q[d]
                cq_p = ps_7.tile([128, NCT + NQT], F32, name="s7")
                for qi in range(NQT):
                    nc.tensor.matmul(cq_p[:, NCT + qi : NCT + qi + 1],
                                     Q_sb.bitcast(F32)[:, qi * 128 : (qi + 1) * 128],
                                     w4Q_sb, start=True, stop=True)
                for ci in range(NCT // 2):
                    nc.tensor.matmul(cq_p[:, ci : ci + 1],
                                     C_sb[:, ci * 128 : (ci + 1) * 128],
                                     w4C_sb, start=True, stop=True)
                bias_c = sb2.tile([128, NCT], F32, name="bias_c")
                nc.vector.tensor_tensor(bias_c[:, 0 : NCT // 2], cq_p[:, 0 : NCT // 2],
                                        cneg[:, 0 : NCT // 2], ALU.add)
                bias_q0 = sb2.tile([128, NQT], F32, name="bias_q0")
                nc.vector.tensor_tensor(bias_q0, cq_p[:, NCT : NCT + NQT], qneg, ALU.add)
                bias_q = sb2.tile([128, NQT], F32, name="bias_q")
                nc.vector.tensor_scalar_add(bias_q, bias_q0, bias_bc[:, 0:1])

                Ct_sb = sb2.tile([128, NCT, 128], BF16, name="Ct_sb")

                def transpose_quad(g):
                    tp = ps_cs.tile([128, 4, 128], F32R, name="cs")
                    for k in range(4):
                        ci = 4 * g + k
                        nc.tensor.matmul(tp[:, k, :], CR[:, ci * 128 : (ci + 1) * 128],
                                         identR, is_transpose=True,
                                         start=(k == 0), stop=(k == 3))
                    nc.vector.tensor_copy(Ct_sb[:, 4 * g : 4 * g + 4, :], tp)

                transpose_quad(0)
                transpose_quad(1)
                transpose_quad(2)
                transpose_quad(3)

                # ---- E2 = exp(S + c-terms) in [c-part, q-free], fused with
                #      R[q,d] += E2[c,q]^T Ct[c,d] and s2sum[q] += colsum ----
                E2 = sb2.tile([128, NCT, LQ], BF16, name="E2")
                rp = ps_r.tile([128, NQT, 128], F32, name="rp")
                s2pp = ps_cs.tile([128, NQT], F32, name="cs")
                for ci in range(NCT):
                    sp = ps_7.tile([128, LQ], F32, name="s7")
                    nc.tensor.matmul(sp, CR[:, ci * 128 : (ci + 1) * 128], Qw,
                                     start=True, stop=True)
                    nc.scalar.activation(E2[:, ci, :], sp, AF.Exp,
                                         bias=bias_c[:, ci : ci + 1], scale=1.0)
                    for qi in range(NQT):
                        lhs = E2[:, ci, qi * 128 : (qi + 1) * 128]
                        first = ci == 0 and qi == 0
                        last = ci == NCT - 1 and qi == NQT - 1
                        nc.tensor.matmul(rp[:, qi, :], lhs, Ct_sb[:, ci, :],
                                         start=first, stop=last)
                        nc.tensor.matmul(s2pp[:, qi : qi + 1], lhs, ones_colB,
                                         start=first, stop=last)
                    if ci == 1:
                        # C-half-1-dependent front matter, emitted late so its
                        # waits don't head-of-line block the PE queue
                        for cil in range(NCT // 2, NCT):
                            nc.tensor.matmul(cq_p[:, cil : cil + 1],
                                             C_sb[:, cil * 128 : (cil + 1) * 128],
                                             w4C_sb, start=True, stop=True)
                        nc.vector.tensor_tensor(bias_c[:, NCT // 2 : NCT],
                                                cq_p[:, NCT // 2 : NCT],
                                                cneg[:, NCT // 2 : NCT], ALU.add)


            # ---- phase 8: E1T = exp(S^T + q-terms) in [q-part, c-free];
            #      per 512-col c-chunk: colsum -> broadcast rcp, A^T, B^T ----
            # R normalize sits here in program order so it doesn't head-of-line
            # block the DVE queue while the rp accumulation is still open.
            rs2 = sb2.tile([128, NQT], F32, name="rs2")
            nc.vector.reciprocal(rs2, s2pp)
            R_sb = sb2.tile([128, NQT, 128], F32R, name="R_sb")
            for qi in range(NQT):
                nc.vector.tensor_scalar_mul(R_sb[:, qi, :], rp[:, qi, :],
                                            rs2[:, qi : qi + 1])
            # C pass-through store: mid-kernel DMA lull, away from loads/tail
            nc.scalar.dma_start(out=out[b, 0:128, :], in_=C_sb)

            E1T = sb2.tile([128, NQT, LC], F32R, name="E1T")

            def consume(cj):
                sl = slice(cj * 512, (cj + 1) * 512)
                csp = ps_ab.tile([128, 512], F32, name="pab")
                for qi in range(NQT):
                    nc.tensor.matmul(csp, onesR128, E1T[:, qi, sl],
                                     start=(qi == 0), stop=(qi == NQT - 1))
                rcp_c = sb4.tile([128, 512], F32, name="rcp_c")
                nc.vector.reciprocal(rcp_c, csp)
                pa = ps_ab.tile([128, 512], F32, name="pab")
                for qi in range(NQT):
                    nc.tensor.matmul(pa, Qt_sb[:, qi, :], E1T[:, qi, sl],
                                     start=(qi == 0), stop=(qi == NQT - 1))
                pb = ps_ab.tile([128, 512], F32, name="pab")
                for qi in range(NQT):
                    nc.tensor.matmul(pb, R_sb[:, qi, :], E1T[:, qi, sl],
                                     start=(qi == 0), stop=(qi == NQT - 1))
                ACB = sb2.tile([128, 3, 512], F32, name="ACB")
                Bt_t = sb2.tile([128, 512], F32, name="Bt_t")
                nc.vector.tensor_tensor(ACB[:, 0, :], pa, rcp_c, ALU.mult)
                nc.vector.tensor_tensor(Bt_t, pb, rcp_c, ALU.mult)
                e1, e2 = (nc.vector, nc.gpsimd) if cj % 2 == 0 else (nc.gpsimd, nc.vector)
                e1.tensor_tensor(ACB[:, 1, :], C_sb[:, sl], ACB[:, 0, :], ALU.mult)
                e2.tensor_tensor(ACB[:, 2, :], C_sb[:, sl], Bt_t, ALU.mult)
                # one DMA stores [At|CA|CB] for this chunk: rows 128:512.
                # Last batch: split so the A rows ship before CA/CB finish.
                if b == BL - 1:
                    nc.sync.dma_start(out=out[b, 128:256, sl], in_=ACB[:, 0, :])
                    nc.sync.dma_start(out=out[b, 256:384, sl], in_=ACB[:, 1, :])
                    nc.sync.dma_start(out=out[b, 384:512, sl], in_=ACB[:, 2, :])
                else:
                    q = nc.sync if cj % 2 == 0 else nc.scalar
                    q.dma_start(
                        out=out[b, 128:512, sl].rearrange("(r p) c -> p r c", p=128),
                        in_=ACB,
                    )

            for cj in range(NCJ):
                sl = slice(cj * 512, (cj + 1) * 512)
                # first chunk's S/exp outranks the phase-7 tail so ACT rolls
                # straight from E2 into E1T exps at the phase boundary
                with (tc.high_priority(P8_BOOST) if cj == 0 else nullcontext()):
                    for qi in range(NQT):
                        sp = ps_8.tile([128, 512], F32, name="s8")
                        nc.tensor.matmul(sp, Qw[:, qi * 128 : (qi + 1) * 128],
                                         CR[:, sl], start=True, stop=True)
                        nc.scalar.activation(E1T[:, qi, sl], sp, AF.Exp,
                                             bias=bias_q[:, qi : qi + 1], scale=1.0)
                if cj > 0:
                    consume(cj - 1)
            consume(NCJ - 1)

    nc.finalize()
    return nc


_NC = None


def _get_nc():
    global _NC
    if _NC is None:
        _NC = _build_nc()
    return _NC


def kernel(C, Q, Cmask, Qmask, w4C, w4Q, w4mlu, bias, _trace=False):
    C = np.ascontiguousarray(np.asarray(C, dtype=np.float32))
    Q = np.ascontiguousarray(np.asarray(Q, dtype=np.float32))
    Cmask = np.ascontiguousarray(np.asarray(Cmask, dtype=np.int32))
    Qmask = np.ascontiguousarray(np.asarray(Qmask, dtype=np.int32))
    w4C = np.ascontiguousarray(np.asarray(w4C, dtype=np.float32))
    w4Q = np.ascontiguousarray(np.asarray(w4Q, dtype=np.float32))
    w4mlu = np.ascontiguousarray(np.asarray(w4mlu, dtype=np.float32))
    bias = np.ascontiguousarray(np.asarray(bias, dtype=np.float32))

    nc = _get_nc()
    in_maps = []
    for i in range(NCORES):
        s = slice(i * BL, (i + 1) * BL)
        in_maps.append({
            "C": C[s], "Q": Q[s], "Cmask": Cmask[s], "Qmask": Qmask[s],
            "w4C": w4C, "w4Q": w4Q, "w4mlu": w4mlu, "bias": bias,
        })
    res = run_bass_kernel_spmd(nc, in_maps, core_ids=list(range(NCORES)),
                               trace=_trace)
    out = np.concatenate([r["out"] for r in res.results], axis=0)
    if _trace:
        kernel._last_results = res
    return out


# revision 42
# speedup vs baseline: 1.1631x; 1.0012x over previous
"""CQAttention Trainium2 kernel.

Reference computation per batch b (C:[D,Lc], Q:[D,Lq], D=128, Lc=2048, Lq=512):
    Ct = C^T, Qt = Q^T
    S  = Ct@w4C + (Qt@w4Q)^T + (Ct*w4mlu)@Qt^T + bias        [Lc, Lq]
    S1 = softmax_q(S + NEG*(1-qmask))                         (over Lq)
    S2 = softmax_c(S + NEG*(1-cmask))                         (over Lc)
    A  = S1 @ Qt                                              [Lc, D]
    B  = S1 @ (S2^T @ Ct)     (right-assoc of (S1 S2^T) Ct)   [Lc, D]
    out= transpose(concat([Ct, A, Ct*A, Ct*B], -1))           [4D, Lc]

Kernel strategy (pure data parallel over batch: 16 batches / 8 cores):
  - S is never materialized: two matmul families compute S(sub2-part) in
    [c-part,q-free] and [q-part,c-free] layouts straight into PSUM; ScalarE
    exp() reads PSUM with a per-partition bias AP folding the softmax-relevant
    affine terms (terms constant along the softmax axis cancel). sub2 uses
    Qw = Q*w4mlu, so C needs no scaled copy (C^T diag(w) Q == C^T (w * Q)).
  - Tile-pipelined phases: each E2=[c,q] exp tile immediately feeds
    R += E2^T@Ct (whole-bank single PSUM accumulation group across all 4
    q-tiles x 16 c-tiles) plus a paired N=1 ones-matmul accumulating the S2
    normalizer in its own bank; PE(S), ACT(exp), PE(R) overlap per c-tile.
  - The S1 normalizer (colsum over q of E1T) is a ones[128,128]-weights
    matmul: every PSUM partition row carries the same colsum, so the
    reciprocal is directly partition-broadcast in SBUF - no DRAM bounce.
  - PSUM pools are phase-private (8 banks: 2 E2-S/transpose/cq, 2 E1T-S/csp,
    2 pa/pb, 1 rp, 1 s2sum) so batch b+1's front (Tile priority boost) never
    rotates behind batch b's tail allocations.
  - PE transposes run as 4-per-bank single accumulation groups (f32r identity,
    1.5 cyc/row) evacuated by one 512-row copy each.
"""

import numpy as np
from contextlib import ExitStack, nullcontext

import concourse.bass as bass
import concourse.mybir as mybir
import concourse.tile as tile
from concourse import bacc
from concourse.bass_utils import run_bass_kernel_spmd
from concourse.masks import make_identity

F32 = mybir.dt.float32
F32R = mybir.dt.float32r
I32 = mybir.dt.int32
AF = mybir.ActivationFunctionType
ALU = mybir.AluOpType
BF16 = mybir.dt.bfloat16

B, D, LC, LQ = 16, 128, 2048, 512
NCORES = 8
BL = B // NCORES          # batches per core
NEG = -1e30
NCT = LC // 128           # 16 c-tiles
NQT = LQ // 128           # 4 q-tiles
NCJ = LC // 512           # 4 c-chunks (free-dim)
HIPRI_OFF = 255
P8_BOOST = 40


def _build_nc():
    nc = bacc.Bacc("TRN2", target_bir_lowering=False)
    Ci = nc.dram_tensor("C", [BL, D, LC], F32, kind="ExternalInput")
    Qi = nc.dram_tensor("Q", [BL, D, LQ], F32, kind="ExternalInput")
    CMi = nc.dram_tensor("Cmask", [BL, LC], I32, kind="ExternalInput")
    QMi = nc.dram_tensor("Qmask", [BL, LQ], I32, kind="ExternalInput")
    w4C = nc.dram_tensor("w4C", [D, 1], F32, kind="ExternalInput")
    w4Q = nc.dram_tensor("w4Q", [D, 1], F32, kind="ExternalInput")
    w4mlu = nc.dram_tensor("w4mlu", [1, 1, D], F32, kind="ExternalInput")
    biasi = nc.dram_tensor("bias", [1], F32, kind="ExternalInput")
    out = nc.dram_tensor("out", [BL, 4 * D, LC], F32, kind="ExternalOutput")

    with tile.TileContext(nc) as tc, ExitStack() as ctx:
        const = ctx.enter_context(tc.tile_pool(name="const", bufs=1))
        sb2 = ctx.enter_context(tc.tile_pool(name="sb2", bufs=2))
        sb4 = ctx.enter_context(tc.tile_pool(name="sb4", bufs=4))
        # PSUM: 8 banks, phase-private pools (see module docstring)
        ps_7 = ctx.enter_context(tc.tile_pool(name="ps_7", bufs=2, space="PSUM"))
        ps_8 = ctx.enter_context(tc.tile_pool(name="ps_8", bufs=2, space="PSUM"))
        ps_ab = ctx.enter_context(tc.tile_pool(name="ps_ab", bufs=2, space="PSUM"))
        ps_r = ctx.enter_context(tc.tile_pool(name="ps_r", bufs=1, space="PSUM"))
        ps_cs = ctx.enter_context(tc.tile_pool(name="ps_cs", bufs=1, space="PSUM"))

        # ---- constants; identity first (Pool), then SWDGE w-consts so the
        #      weight loads don't sit behind identity building on Pool ----
        ident0 = const.tile([D, D], F32, name="ident0")
        make_identity(nc, ident0)
        identR = const.tile([D, D], F32R, name="identR")
        nc.vector.tensor_copy(identR, ident0)
        ones_f = const.tile([D, D], F32, name="ones_f")
        nc.vector.memset(ones_f, 1.0)
        onesR128 = const.tile([D, D], F32R, name="onesR128")
        nc.vector.tensor_copy(onesR128, ones_f)
        ones_colB = const.tile([D, 1], BF16, name="ones_colB")
        nc.vector.tensor_copy(ones_colB, ones_f[:, 0:1])
        wmlu_sb = const.tile([D, 1], F32, name="wmlu_sb")
        nc.gpsimd.dma_start(out=wmlu_sb, in_=w4mlu.ap().rearrange("a b d -> d (a b)"))
        w4C_sb = const.tile([D, 1], F32, name="w4C_sb")
        nc.gpsimd.dma_start(out=w4C_sb, in_=w4C[:, :])
        w4Q_sb = const.tile([D, 1], F32, name="w4Q_sb")
        nc.gpsimd.dma_start(out=w4Q_sb, in_=w4Q[:, :])
        bias_bc = const.tile([D, 1], F32, name="bias_bc")
        nc.scalar.dma_start(out=bias_bc, in_=biasi.ap().partition_broadcast(D))

        for b in range(BL):
            with (tc.high_priority(HIPRI_OFF) if b > 0 else nullcontext()):
                # ---- loads: Q, C-half0, masks, C-half1 ----
                Q_sb = sb2.tile([D, LQ], F32R, name="Q_sb")
                nc.sync.dma_start(out=Q_sb, in_=Qi[b, :, :].bitcast(F32R))
                C_sb = sb2.tile([D, LC], F32, name="C_sb")
                nc.sync.dma_start(out=C_sb[:, 0:512], in_=Ci[b, :, 0:512])
                cm_i = sb2.tile([128, NCT], I32, name="cm_i")
                nc.sync.dma_start(out=cm_i, in_=CMi[b, :].rearrange("(i p) -> p i", p=128))
                nc.sync.dma_start(out=C_sb[:, 512:1024], in_=Ci[b, :, 512:1024])
                qm_i = sb2.tile([128, NQT], I32, name="qm_i")
                nc.sync.dma_start(out=qm_i, in_=QMi[b, :].rearrange("(i p) -> p i", p=128))
                nc.sync.dma_start(out=C_sb[:, 1024:2048], in_=Ci[b, :, 1024:2048])

                # ---- small prep ----
                Qw = sb2.tile([D, LQ], F32R, name="Qw")
                nc.vector.tensor_scalar_mul(Qw, Q_sb, wmlu_sb[:, 0:1])
                # mask -> NEG*(1-m):  m*(-NEG) + NEG
                cneg = sb2.tile([128, NCT], F32, name="cneg")
                nc.vector.tensor_scalar(cneg, cm_i, -NEG, NEG, op0=ALU.mult, op1=ALU.add)
                qneg = sb2.tile([128, NQT], F32, name="qneg")
                nc.vector.tensor_scalar(qneg, qm_i, -NEG, NEG, op0=ALU.mult, op1=ALU.add)
                # f32r matmul operands must be pre-rounded to f32r in SBUF;
                # chunk 0 on the (idle-early) ACT engine so the first S-matmul
                # isn't gated by Pool's const DMAs
                CR = sb2.tile([D, LC], F32R, name="CR")
                for cj in range(NCJ):
                    sl0 = slice(cj * 512, (cj + 1) * 512)
                    if cj == 0:
                        nc.scalar.copy(CR[:, sl0], C_sb[:, sl0])
                    else:
                        eng = nc.gpsimd if cj % 2 == 0 else nc.vector
                        eng.tensor_copy(CR[:, sl0], C_sb[:, sl0])

                # ---- Q-only transposes first (C loads still in flight) ----
                Qt_sb = sb2.tile([128, NQT, 128], F32R, name="Qt_sb")
                tpq = ps_cs.tile([128, NQT, 128], F32R, name="cs")
                for qi in range(NQT):
                    nc.tensor.matmul(tpq[:, qi, :], Q_sb[:, qi * 128 : (qi + 1) * 128],
                                     identR, is_transpose=True,
                                     start=(qi == 0), stop=(qi == NQT - 1))
                nc.vector.tensor_copy(Qt_sb, tpq)

                # c0[c] = sum_d C[d,c] w4C[d]; q1[q] = sum_d Q[d,q] w4Q[d]
                cq_p = ps_7.tile([128, NCT + NQT], F32, name="s7")
                for qi in range(NQT):
                    nc.tensor.matmul(cq_p[:, NCT + qi : NCT + qi + 1],
                                     Q_sb.bitcast(F32)[:, qi * 128 : (qi + 1) * 128],
                                     w4Q_sb, start=True, stop=True)
                for ci in range(NCT // 2):
                    nc.tensor.matmul(cq_p[:, ci : ci + 1],
                                     C_sb[:, ci * 128 : (ci + 1) * 128],
                                     w4C_sb, start=True, stop=True)
                bias_c = sb2.tile([128, NCT], F32, name="bias_c")
                nc.vector.tensor_tensor(bias_c[:, 0 : NCT // 2], cq_p[:, 0 : NCT // 2],
                                        cneg[:, 0 : NCT // 2], ALU.add)
                bias_q0 = sb2.tile([128, NQT], F32, name="bias_q0")
                nc.vector.tensor_tensor(bias_q0, cq_p[:, NCT : NCT + NQT], qneg, ALU.add)
                bias_q = sb2.tile([128, NQT], F32, name="bias_q")
                nc.vector.tensor_scalar_add(bias_q, bias_q0, bias_bc[:, 0:1])

                Ct_sb = sb2.tile([128, NCT, 128], BF16, name="Ct_sb")

                def transpose_quad(g):
                    tp = ps_cs.tile([128, 4, 128], F32R, name="cs")
                    for k in range(4):
                        ci = 4 * g + k
                        nc.tensor.matmul(tp[:, k, :], CR[:, ci * 128 : (ci + 1) * 128],
                                         identR, is_transpose=True,
                                         start=(k == 0), stop=(k == 3))
                    nc.vector.tensor_copy(Ct_sb[:, 4 * g : 4 * g + 4, :], tp)

                transpose_quad(0)
                transpose_quad(1)
                transpose_quad(2)
                transpose_quad(3)

                # ---- E2 = exp(S + c-terms) in [c-part, q-free], fused with
                #      R[q,d] += E2[c,q]^T Ct[c,d] and s2sum[q] += colsum ----
                E2 = sb2.tile([128, NCT, LQ], BF16, name="E2")
                rp = ps_r.tile([128, NQT, 128], F32, name="rp")
                s2pp = ps_cs.tile([128, NQT], F32, name="cs")
                for ci in range(NCT):
                    sp = ps_7.tile([128, LQ], F32, name="s7")
                    nc.tensor.matmul(sp, CR[:, ci * 128 : (ci + 1) * 128], Qw,
                                     start=True, stop=True)
                    nc.scalar.activation(E2[:, ci, :], sp, AF.Exp,
                                         bias=bias_c[:, ci : ci + 1], scale=1.0)
                    for qi in range(NQT):
                        lhs = E2[:, ci, qi * 128 : (qi + 1) * 128]
                        first = ci == 0 and qi == 0
                        last = ci == NCT - 1 and qi == NQT - 1
                        nc.tensor.matmul(rp[:, qi, :], lhs, Ct_sb[:, ci, :],
                                         start=first, stop=last)
                        nc.tensor.matmul(s2pp[:, qi : qi + 1], lhs, ones_colB,
                                         start=first, stop=last)
                    if ci == 1:
                        # C-half-1-dependent front matter, emitted late so its
                        # waits don't head-of-line block the PE queue
                        for cil in range(NCT // 2, NCT):
                            nc.tensor.matmul(cq_p[:, cil : cil + 1],
                                             C_sb[:, cil * 128 : (cil + 1) * 128],
                                             w4C_sb, start=True, stop=True)
                        nc.vector.tensor_tensor(bias_c[:, NCT // 2 : NCT],
                                                cq_p[:, NCT // 2 : NCT],
                                                cneg[:, NCT // 2 : NCT], ALU.add)


            # ---- phase 8: E1T = exp(S^T + q-terms) in [q-part, c-free];
            #      per 512-col c-chunk: colsum -> broadcast rcp, A^T, B^T ----
            # R normalize sits here in program order so it doesn't head-of-line
            # block the DVE queue while the rp accumulation is still open.
            rs2 = sb2.tile([128, NQT], F32, name="rs2")
            nc.vector.reciprocal(rs2, s2pp)
            R_sb = sb2.tile([128, NQT, 128], F32R, name="R_sb")
            for qi in range(NQT):
                nc.vector.tensor_scalar_mul(R_sb[:, qi, :], rp[:, qi, :],
                                            rs2[:, qi : qi + 1])
            # C pass-through store: mid-kernel DMA lull, away from loads/tail
            nc.scalar.dma_start(out=out[b, 0:128, :], in_=C_sb)

            E1T = sb2.tile([128, NQT, LC], F32R, name="E1T")

            def consume(cj):
                sl = slice(cj * 512, (cj + 1) * 512)
                csp = ps_ab.tile([128, 512], F32, name="pab")
                for qi in range(NQT):
                    nc.tensor.matmul(csp, onesR128, E1T[:, qi, sl],
                                     start=(qi == 0), stop=(qi == NQT - 1))
                rcp_c = sb4.tile([128, 512], F32, name="rcp_c")
                nc.vector.reciprocal(rcp_c, csp)
                pa = ps_ab.tile([128, 512], F32, name="pab")
                for qi in range(NQT):
                    nc.tensor.matmul(pa, Qt_sb[:, qi, :], E1T[:, qi, sl],
                                     start=(qi == 0), stop=(qi == NQT - 1))
                pb = ps_ab.tile([128, 512], F32, name="pab")
                for qi in range(NQT):
                    nc.tensor.matmul(pb, R_sb[:, qi, :], E1T[:, qi, sl],
                                     start=(qi == 0), stop=(qi == NQT - 1))
                ACB = sb2.tile([128, 3, 512], F32, name="ACB")
                Bt_t = sb2.tile([128, 512], F32, name="Bt_t")
                nc.vector.tensor_tensor(ACB[:, 0, :], pa, rcp_c, ALU.mult)
                nc.vector.tensor_tensor(Bt_t, pb, rcp_c, ALU.mult)
                e1, e2 = (nc.vector, nc.gpsimd) if cj % 2 == 0 else (nc.gpsimd, nc.vector)
                e1.tensor_tensor(ACB[:, 1, :], C_sb[:, sl], ACB[:, 0, :], ALU.mult)
                e2.tensor_tensor(ACB[:, 2, :], C_sb[:, sl], Bt_t, ALU.mult)
                # one DMA stores [At|CA|CB] for this chunk: rows 128:512.
                # Last batch: split so the A rows ship before CA/CB finish.
                if b == BL - 1:
                    nc.sync.dma_start(out=out[b, 128:256, sl], in_=ACB[:, 0, :])
                    nc.sync.dma_start(out=out[b, 256:384, sl], in_=ACB[:, 1, :])
                    nc.sync.dma_start(out=out[b, 384:512, sl], in_=ACB[:, 2, :])
                else:
                    q = nc.sync if cj % 2 == 0 else nc.scalar
                    q.dma_start(
                        out=out[b, 128:512, sl].rearrange("(r p) c -> p r c", p=128),
                        in_=ACB,
                    )

            for cj in range(NCJ):
                sl = slice(cj * 512, (cj + 1) * 512)
                # first chunk's S/exp outranks the phase-7 tail so ACT rolls
                # straight from E2 into E1T exps at the phase boundary
                with (tc.high_priority(P8_BOOST) if cj == 0 else nullcontext()):
                    for qi in range(NQT):
                        sp = ps_8.tile([128, 512], F32, name="s8")
                        nc.tensor.matmul(sp, Qw[:, qi * 128 : (qi + 1) * 128],
                                         CR[:, sl], start=True, stop=True)
                        nc.scalar.activation(E1T[:, qi, sl], sp, AF.Exp,
                                             bias=bias_q[:, qi : qi + 1], scale=1.0)
                if cj > 0:
                    consume(cj - 1)
            consume(NCJ - 1)

    nc.finalize()
    return nc


_NC = None


def _get_nc():
    global _NC
    if _NC is None:
        _NC = _build_nc()
    return _NC


def kernel(C, Q, Cmask, Qmask, w4C, w4Q, w4mlu, bias, _trace=False):
    C = np.ascontiguousarray(np.asarray(C, dtype=np.float32))
    Q = np.ascontiguousarray(np.asarray(Q, dtype=np.float32))
    Cmask = np.ascontiguousarray(np.asarray(Cmask, dtype=np.int32))
    Qmask = np.ascontiguousarray(np.asarray(Qmask, dtype=np.int32))
    w4C = np.ascontiguousarray(np.asarray(w4C, dtype=np.float32))
    w4Q = np.ascontiguousarray(np.asarray(w4Q, dtype=np.float32))
    w4mlu = np.ascontiguousarray(np.asarray(w4mlu, dtype=np.float32))
    bias = np.ascontiguousarray(np.asarray(bias, dtype=np.float32))

    nc = _get_nc()
    in_maps = []
    for i in range(NCORES):
        s = slice(i * BL, (i + 1) * BL)
        in_maps.append({
            "C": C[s], "Q": Q[s], "Cmask": Cmask[s], "Qmask": Qmask[s],
            "w4C": w4C, "w4Q": w4Q, "w4mlu": w4mlu, "bias": bias,
        })
    res = run_bass_kernel_spmd(nc, in_maps, core_ids=list(range(NCORES)),
                               trace=_trace)
    out = np.concatenate([r["out"] for r in res.results], axis=0)
    if _trace:
        kernel._last_results = res
    return out
